# revision 3
# baseline (speedup 1.0000x reference)
"""GAT (3-layer, 4/4/1 heads) on 8 TRN2 NeuronCores.

Design (per core, SPMD one program):
- Nodes dealt to cores by degree (snake) for edge balance; within a core,
  nodes are 2D-packed by (deg_A, deg_B) into 128-node blocks; slot grid per
  block: [128 nodes x T slots], split into A/B src-half grids (int16
  dma_gather reach).
- Record tables per layer in DRAM: [feat | el | er | pad] per node, built by
  a dense phase (PE) on owned nodes, AllGathered to every core.
- Edge phase per block: dma_gather of src records (4 SWDGE queues), then
  e = el_src + er_dst(bcast) -> +mask -> leaky-relu -> exp (ACT) -> denom
  reduce -> alpha -> msg = feat * alpha (DVE) -> slot-reduce -> bias+relu.
- Next layer's dense phase: PE transpose x -> matmul with [W^T|wal|war].
"""

import os
import numpy as np

import concourse.bacc as bacc
import concourse.bass as bass
import concourse.tile as tile
from concourse import mybir
from concourse.masks import make_identity

F32 = mybir.dt.float32
I16 = mybir.dt.int16

NEG = np.float32(-1e30)


# ---------------------------------------------------------------- host prep

def prep_host(src, dst, N, n_cores=8, blk=128):
    """Graph -> per-core slot grids. Pure index manipulation.

    Returns dict with per-core device arrays and the shared static structure.
    """
    src = np.asarray(src); dst = np.asarray(dst)
    deg = np.bincount(dst, minlength=N)

    # snake-deal nodes by degree to balance per-core edge counts
    order = np.argsort(-deg, kind="stable")
    owner = np.empty(N, np.int32)
    for i, v in enumerate(order):
        r = i % (2 * n_cores)
        owner[v] = r if r < n_cores else 2 * n_cores - 1 - r

    half_of_core = (np.arange(n_cores) >= n_cores // 2).astype(np.int32)

    # deg_A/deg_B per node (A = src owned by cores 0..3)
    src_half = half_of_core[owner[src]]
    degA = np.bincount(dst[src_half == 0], minlength=N)
    degB = np.bincount(dst[src_half == 1], minlength=N)

    core_nodes = [np.where(owner == c)[0] for c in range(n_cores)]
    npc_real = max(len(cn) for cn in core_nodes)
    assert all(len(cn) == npc_real for cn in core_nodes), "need uniform per-core node count"
    # always at least one pad row per core (the gather pad-row target)
    nblk = (npc_real + blk) // blk
    npc = nblk * blk

    # within each core: 2D pack by (degA, degB), then blocks of `blk`
    layouts = []   # per core: node id per row (-1 = pad)
    for c in range(n_cores):
        cn = core_nodes[c]
        key = np.maximum(degA[cn], degB[cn]) * 1000 + np.minimum(degA[cn], degB[cn])
        cn = cn[np.argsort(-key, kind="stable")]
        lay = np.full(npc, -1, np.int64)
        lay[: len(cn)] = cn
        layouts.append(lay)

    # per-core per-block T_A/T_B, canonical block order by total T desc
    TA = np.zeros((n_cores, nblk), np.int32)
    TB = np.zeros((n_cores, nblk), np.int32)
    for c in range(n_cores):
        for b in range(nblk):
            rows = layouts[c][b * blk:(b + 1) * blk]
            real = rows[rows >= 0]
            TA[c, b] = degA[real].max() if len(real) else 0
            TB[c, b] = degB[real].max() if len(real) else 0
    # sort each core's blocks by (TA+TB) desc, then reorder layout rows
    for c in range(n_cores):
        bo = np.argsort(-(TA[c] + TB[c]), kind="stable")
        layouts[c] = np.concatenate(
            [layouts[c][b * blk:(b + 1) * blk] for b in bo])
        TA[c], TB[c] = TA[c][bo], TB[c][bo]

    # shared profile (max across cores); keep per-block total even
    TAs = TA.max(axis=0)
    TBs = TB.max(axis=0)
    TBs = TBs + ((TAs + TBs) % 2)

    # block processing order: pair largest with smallest
    pairs = []
    lo, hi = 0, nblk - 1
    while lo < hi:
        pairs.append((hi, lo))   # (big, small)
        lo += 1; hi -= 1
    if lo == hi:
        pairs.append((lo,))
    # pairs above pair big..small by T desc order: blocks sorted desc, so
    # block 0 is biggest -> pair (0, nblk-1), (1, nblk-2) ...
    # biggest block runs alone (caps the grid pool slot size); rest paired
    pairs = [(0,)]
    lo, hi = 1, nblk - 1
    while lo <= hi:
        pairs.append((lo, hi) if lo != hi else (lo,))
        lo += 1; hi -= 1

    # pi: global row of node v
    pi = np.full(N, -1, np.int64)
    for c in range(n_cores):
        rows = layouts[c]
        valid = rows >= 0
        pi[rows[valid]] = c * npc + np.where(valid)[0]
    assert (pi[np.unique(dst)] >= 0).all()
    half_rows = (n_cores // 2) * npc   # A-half row count

    # in-edge lists per node, split by half
    eorder = np.argsort(dst, kind="stable")
    esrc = src[eorder]; edst = dst[eorder]
    estart = np.searchsorted(edst, np.arange(N))
    eend = np.searchsorted(edst, np.arange(N), side="right")

    # grid column layout: processing order = for pair -> for blk -> A then B
    col_T = []        # (core-agnostic) list of (blk, half, T) in order
    for pr in pairs:
        for b in pr:
            col_T.append((b, 0, int(TAs[b])))
            col_T.append((b, 1, int(TBs[b])))
    tot_T = sum(t for _, _, t in col_T)

    idx_mega = np.zeros((n_cores, 128, 8 * tot_T), np.int16)

    for c in range(n_cores):
        lay = layouts[c]
        off = 0
        for (b, half, T) in col_T:
            if T == 0:
                continue
            iv = np.full((128, T), half_rows - 1, np.int64)   # pad -> pad row
            rows = lay[b * blk:(b + 1) * blk]
            for p in range(blk):
                v = rows[p]
                if v < 0:
                    if half == 0:
                        # give pad nodes one real slot: denom > 0, no NaNs
                        iv[p, 0] = 0
                    continue
                es = esrc[estart[v]:eend[v]]
                hs = half_of_core[owner[es]]
                mysrc = es[hs == half]
                k = len(mysrc)
                assert k <= T
                r = pi[mysrc] - half * half_rows
                assert (r >= 0).all() and (r < half_rows).all()
                iv[p, :k] = r
            # wrap into dma_gather idx layout: pos i=(j*128+p) -> [i%16, i//16]
            pos = np.arange(128 * T)
            wrapped = np.zeros((16, 8 * T), np.int16)
            vals = iv[pos % 128, pos // 128]
            assert vals.max() < 32768
            wrapped[pos % 16, pos // 16] = vals.astype(np.int16)
            idx_mega[c, :, off * 8:(off + T) * 8] = np.tile(wrapped, (8, 1))
            off += T
        assert off == tot_T

    return dict(
        n_cores=n_cores, blk=blk, nblk=nblk, npc=npc, n_real=npc_real,
        half_rows=half_rows, pairs=pairs, col_T=col_T, tot_T=tot_T,
        TAs=TAs, TBs=TBs, layouts=layouts, pi=pi, owner=owner,
        idx_mega=idx_mega,
    )


# ------------------------------------------------------------ device build

def build_program(st, H=4, DH=32, D_IN=128, D_OUT=64, nq=4):
    """Build the SPMD Bass program (same for all cores)."""
    n_cores, blk, nblk, npc = st["n_cores"], st["blk"], st["nblk"], st["npc"]
    tot_T, col_T, pairs = st["tot_T"], st["col_T"], st["pairs"]
    HD = H * DH                      # 128
    REC = 192                        # layer-0/1 record floats
    REC2 = 128                       # layer-2 record floats
    NROWS = n_cores * npc
    half_rows = st["half_rows"]

    nc = bacc.Bacc(None, target_bir_lowering=False, num_swdge_queues=nq)

    feats_own = nc.dram_tensor("feats_own", [npc, D_IN], F32, kind="ExternalInput")
    idx_in = nc.dram_tensor("idx_mega", [128, 8 * tot_T], I16, kind="ExternalInput")
    W0_in = nc.dram_tensor("W0", [HD, D_IN], F32, kind="ExternalInput")
    W1_in = nc.dram_tensor("W1", [HD, HD], F32, kind="ExternalInput")
    W2_in = nc.dram_tensor("W2", [D_OUT, HD], F32, kind="ExternalInput")
    M0_in = nc.dram_tensor("M0", [HD, HD + 8], F32, kind="ExternalInput")
    M1_in = nc.dram_tensor("M1", [HD, HD + 8], F32, kind="ExternalInput")
    M2_in = nc.dram_tensor("M2", [D_OUT, D_OUT + 2], F32, kind="ExternalInput")
    b0_in = nc.dram_tensor("b0", [1, HD], F32, kind="ExternalInput")
    b1_in = nc.dram_tensor("b1", [1, HD], F32, kind="ExternalInput")
    b2_in = nc.dram_tensor("b2", [1, D_OUT], F32, kind="ExternalInput")
    ones_in = nc.dram_tensor("ones", [1, 128], F32, kind="ExternalInput")
    out_t = nc.dram_tensor("out", [npc, D_OUT], F32, kind="ExternalOutput")

    LCFG = [  # per layer: record floats, feat width, heads, el col, er col
        dict(rec=REC, fw=HD, h=H, elc=HD, erc=HD + 4, wcat=HD + 8),
        dict(rec=REC, fw=HD, h=H, elc=HD, erc=HD + 4, wcat=HD + 8),
        dict(rec=REC2, fw=D_OUT, h=1, elc=D_OUT, erc=D_OUT + 1, wcat=D_OUT + 2),
    ]

    with tile.TileContext(nc) as tc:
        with tc.tile_pool(name="const", bufs=1) as cpool, \
             tc.tile_pool(name="dram", bufs=1, space="DRAM") as dram, \
             tc.tile_pool(name="xblk", bufs=1) as xpool, \
             tc.tile_pool(name="gat", bufs=2) as gpool, \
             tc.tile_pool(name="msg", bufs=3) as mpool, \
             tc.tile_pool(name="small", bufs=4) as spool, \
             tc.tile_pool(name="psum", bufs=2, space="PSUM") as ppool, \
             tc.tile_pool(name="psum1", bufs=2, space="PSUM") as ppool2:

            # ---------------- constants / prologue
            idx_sb = cpool.tile([128, 8 * tot_T], I16)
            nc.sync.dma_start(out=idx_sb[:], in_=idx_in[:])
            ident = cpool.tile([128, 128], F32)
            make_identity(nc, ident[:])
            # -1e30 tile: overwrites el of the per-core pad rows each layer
            pad_el = cpool.tile([blk, 8], F32)
            nc.vector.memset(pad_el[:], NEG)

            wins = [W0_in, W1_in, W2_in]
            mins = [M0_in, M1_in, M2_in]
            bins = [b0_in, b1_in, b2_in]
            wcat_sb, brep_sb = [], []
            with tc.tile_pool(name="prolog", bufs=1) as plpool:
                ones_sb = plpool.tile([1, 128], F32)
                nc.sync.dma_start(out=ones_sb[:], in_=ones_in[:])
                for L in range(3):
                    cfg = LCFG[L]
                    kdim = wins[L].shape[0]
                    w_sb = plpool.tile([kdim, wins[L].shape[1]], F32, tag=f"wld{L}", name=f"wld{L}")
                    nc.sync.dma_start(out=w_sb[:], in_=wins[L][:])
                    m_sb = plpool.tile([kdim, cfg["wcat"]], F32, tag=f"mld{L}", name=f"mld{L}")
                    nc.sync.dma_start(out=m_sb[:], in_=mins[L][:])
                    wc_ps = ppool.tile([wins[L].shape[1], cfg["wcat"]], F32, space="PSUM", tag="wcps")
                    nc.tensor.matmul(out=wc_ps[:], lhsT=w_sb[:], rhs=m_sb[:], start=True, stop=True)
                    wc = cpool.tile([wins[L].shape[1], cfg["wcat"]], F32, tag=f"wcat{L}", name=f"wcat{L}")
                    nc.vector.tensor_copy(out=wc[:], in_=wc_ps[:])
                    wcat_sb.append(wc)

                    b_sb = plpool.tile([1, cfg["fw"]], F32, tag=f"bld{L}", name=f"bld{L}")
                    nc.sync.dma_start(out=b_sb[:], in_=bins[L][:])
                    br_ps = ppool.tile([128, cfg["fw"]], F32, space="PSUM", tag="brps")
                    nc.tensor.matmul(out=br_ps[:], lhsT=ones_sb[:], rhs=b_sb[:], start=True, stop=True)
                    br = cpool.tile([128, cfg["fw"]], F32, tag=f"brep{L}", name=f"brep{L}")
                    nc.vector.tensor_copy(out=br[:], in_=br_ps[:])
                    brep_sb.append(br)

            # persistent per-block x tiles and er tiles
            x_tiles = [xpool.tile([128, HD], F32, tag=f"x{b}", name=f"xblk{b}") for b in range(nblk)]
            er_tiles = [xpool.tile([128, H], F32, tag=f"er{b}", name=f"erblk{b}") for b in range(nblk)]
            for b in range(nblk):
                if st["TAs"][b] + st["TBs"][b] == 0:
                    # all-pad block: edge phase never writes it
                    nc.vector.memset(x_tiles[b][:], 0)

            # record staging tiles (manually double-buffered so the pad
            # columns can be zeroed exactly once)
            rec_tiles = [xpool.tile([128, REC], F32, tag=f"rec{i}", name=f"rect{i}") for i in range(2)]
            for i in range(2):
                nc.vector.memset(rec_tiles[i][:], 0)

            # DRAM slabs + AG tables
            slabs = [dram.tile([npc, REC], F32, tag="slab0", name="slab0"),
                     dram.tile([npc, REC], F32, tag="slab1", name="slab1"),
                     dram.tile([npc, REC2], F32, tag="slab2", name="slab2")]
            tables = [dram.tile([NROWS, REC], F32, tag="tab0", name="tab0"),
                      dram.tile([NROWS, REC], F32, tag="tab1", name="tab1"),
                      dram.tile([NROWS, REC2], F32, tag="tab2", name="tab2")]

            # column offsets of each (blk, half) range in the grids
            col_off = {}
            off = 0
            for (b, half, T) in col_T:
                col_off[(b, half)] = (off, T)
                off += T

            # ---------------- per layer
            n_real = st["n_real"]

            def dense_block(L, b):
                """Record rows for owned block b of layer L (PE + copies)."""
                cfg = LCFG[L]
                if L == 0:
                    x_sb = spool.tile([128, D_IN], F32, tag="xts", name="x0ld")
                    nc.sync.dma_start(out=x_sb[:], in_=feats_own[b * blk:(b + 1) * blk, :])
                else:
                    x_sb = x_tiles[b]
                xt_ps = ppool.tile([128, 128], F32, space="PSUM", tag="xtps", name="xt_ps")
                nc.tensor.transpose(out=xt_ps[:], in_=x_sb[:], identity=ident[:])
                xt_sb = spool.tile([128, 128], F32, tag="xts", name="xt_sb")
                if b % 2 == 0:
                    nc.scalar.copy(out=xt_sb[:], in_=xt_ps[:])
                else:
                    nc.vector.tensor_copy(out=xt_sb[:], in_=xt_ps[:])
                y_ps = ppool2.tile([128, cfg["wcat"]], F32, space="PSUM", tag="yps", name="y_ps")
                nc.tensor.matmul(out=y_ps[:], lhsT=xt_sb[:], rhs=wcat_sb[L][:], start=True, stop=True)
                recb = rec_tiles[b % 2]
                nc.scalar.copy(out=recb[:, 0:cfg["wcat"]], in_=y_ps[:])
                nc.vector.tensor_copy(out=er_tiles[b][:, 0:cfg["h"]],
                                      in_=y_ps[:, cfg["erc"]:cfg["erc"] + cfg["h"]])
                nc.sync.dma_start(out=slabs[L][b * blk:(b + 1) * blk, :],
                                  in_=recb[:, 0:cfg["rec"]])

            def finish_slab(L):
                """Pad-row el poison + AllGather into the layer table."""
                cfg = LCFG[L]
                nc.sync.dma_start(out=slabs[L][n_real:npc, cfg["elc"]:cfg["elc"] + cfg["h"]],
                                  in_=pad_el[0:npc - n_real, 0:cfg["h"]])
                nc.gpsimd.collective_compute(
                    "AllGather", mybir.AluOpType.bypass,
                    replica_groups=[list(range(n_cores))],
                    ins=[slabs[L].opt()], outs=[tables[L].opt()],
                )

            for b in range(nblk):
                dense_block(0, b)
            finish_slab(0)

            for L in range(3):
                cfg = LCFG[L]
                h, fw, rec = cfg["h"], cfg["fw"], cfg["rec"]
                elc, erc = cfg["elc"], cfg["erc"]
                tabA = tables[L][0:half_rows, :]
                tabB = tables[L][half_rows:NROWS, :]

                # edge phase, per block pair
                gcount = 0
                for pi_, pr in enumerate(pairs):
                    Ts = [col_off[(b, hf)][1] for b in pr for hf in (0, 1)]
                    Tg = sum(Ts)
                    if Tg == 0:
                        continue
                    grid = gpool.tile([128, Tg, rec], F32, tag="grid")
                    # gathers: <=1024 idxs each (single_packet needs <=64
                    # descs per engine), spread over the 4 SWDGE queues
                    # (queue = (count%8)//2 keeps Tile's DMASW lane rotation
                    # queue-pure: lane k always serves queue k//2)
                    goff = 0
                    for b in pr:
                        for hf in (0, 1):
                            o, T = col_off[(b, hf)]
                            for t0 in range(0, T, 8):
                                tw = min(8, T - t0)
                                nc.gpsimd.dma_gather(
                                    out_ap=grid[:, goff + t0:goff + t0 + tw, :],
                                    in_ap=(tabA if hf == 0 else tabB),
                                    idxs_ap=idx_sb[:, (o + t0) * 8:(o + t0 + tw) * 8],
                                    num_idxs=128 * tw,
                                    num_idxs_reg=128 * tw,
                                    elem_size=rec,
                                    single_packet=True,
                                    queue_num=(gcount % (2 * nq)) // 2,
                                )
                                gcount += 1
                            goff += T

                    goff = 0
                    for b in pr:
                        _, TA_ = col_off[(b, 0)]
                        _, TB_ = col_off[(b, 1)]
                        T = TA_ + TB_
                        g = grid[:, goff:goff + T, :]
                        goff += T
                        if T == 0:
                            continue
                        # e = el + er (bcast over slots), in place on el cols
                        el = g[:, :, elc:elc + h]                     # [128,T,h]
                        erb = er_tiles[b][:, 0:h].unsqueeze(1).to_broadcast([128, T, h])
                        nc.vector.tensor_tensor(out=el, in0=el, in1=erb, op=mybir.AluOpType.add)
                        # leaky relu (pads arrive as el=-1e30 -> exp gives 0)
                        nc.vector.scalar_tensor_tensor(
                            out=el, in0=el, scalar=0.2, in1=el,
                            op0=mybir.AluOpType.mult, op1=mybir.AluOpType.max)
                        # ex = exp(e)
                        nc.scalar.activation(out=el, in_=el, func=mybir.ActivationFunctionType.Exp)
                        # unnormalized msg[p, h, d, t] = feat * ex
                        dh = fw // h
                        msg = mpool.tile([128, fw, Tg], F32, tag="msg")
                        feat_v = g[:, :, 0:fw].rearrange("p t (h d) -> p h d t", h=h)
                        ex_v = g[:, :, elc:elc + h].rearrange("p t h -> p h t").unsqueeze(2).to_broadcast([128, h, dh, T])
                        msg_v = msg[:, 0:fw, 0:T].rearrange("p (h d) t -> p h d t", h=h)
                        nc.vector.tensor_tensor(out=msg_v, in0=feat_v, in1=ex_v, op=mybir.AluOpType.mult)
                        # denominator (+eps against all-pad rows), reciprocal
                        den = spool.tile([128, h], F32, tag="den")
                        elT = g[:, :, elc:elc + h].rearrange("p t h -> p h t")
                        nc.vector.tensor_reduce(out=den[:], in_=elT, axis=mybir.AxisListType.X,
                                                op=mybir.AluOpType.add)
                        rden = spool.tile([128, h], F32, tag="rden")
                        nc.vector.reciprocal(out=rden[:], in_=den[:])
                        # agg over slots (unnormalized)
                        agg = spool.tile([128, fw], F32, tag="agg")
                        nc.vector.tensor_reduce(out=agg[:], in_=msg[:, 0:fw, 0:T],
                                                axis=mybir.AxisListType.X, op=mybir.AluOpType.add)
                        # finalize: x = (relu of) agg * rden + bias
                        rdb = rden[:, 0:h].unsqueeze(2).to_broadcast([128, h, dh])
                        agg_v = agg[:].rearrange("p (h d) -> p h d", h=h)
                        nc.vector.tensor_tensor(out=agg_v, in0=agg_v, in1=rdb, op=mybir.AluOpType.mult)
                        if L < 2:
                            nc.vector.tensor_tensor(out=agg[:], in0=agg[:], in1=brep_sb[L][:],
                                                    op=mybir.AluOpType.add)
                            # relu on the (otherwise idle) scalar engine
                            nc.scalar.activation(out=x_tiles[b][:], in_=agg[:],
                                                 func=mybir.ActivationFunctionType.Relu)
                            # overlap next layer's dense work with this edge phase
                            dense_block(L + 1, b)
                        else:
                            ob = spool.tile([128, D_OUT], F32, tag="ob")
                            nc.vector.tensor_tensor(out=ob[:], in0=agg[:], in1=brep_sb[2][:],
                                                    op=mybir.AluOpType.add)
                            nc.sync.dma_start(out=out_t[b * blk:(b + 1) * blk, :], in_=ob[:])

                if L < 2:
                    # all-pad blocks never hit the edge loop: dense them here
                    for b in range(nblk):
                        if st["TAs"][b] + st["TBs"][b] == 0:
                            dense_block(L + 1, b)
                    finish_slab(L + 1)

    nc.compile()
    return nc


# ------------------------------------------------------------------- runner

def make_in_maps(st, inputs, H=4, DH=32, D_IN=128, D_OUT=64):
    n_cores, npc = st["n_cores"], st["npc"]
    feats = np.asarray(inputs["feats"], np.float32)
    HD = H * DH

    def mk_M(al, ar, d):
        h = al.shape[0]
        M = np.zeros((d, d + 2 * h), np.float32)
        M[:, :d] = np.eye(d, dtype=np.float32)
        for hh in range(h):
            M[hh * (d // h):(hh + 1) * (d // h), d + hh] = al[hh]
            M[hh * (d // h):(hh + 1) * (d // h), d + h + hh] = ar[hh]
        return M

    M0 = mk_M(np.asarray(inputs["al0"]), np.asarray(inputs["ar0"]), HD)
    M1 = mk_M(np.asarray(inputs["al1"]), np.asarray(inputs["ar1"]), HD)
    M2 = mk_M(np.asarray(inputs["al2"]), np.asarray(inputs["ar2"]), D_OUT)

    shared = dict(
        W0=np.asarray(inputs["W0"], np.float32),
        W1=np.asarray(inputs["W1"], np.float32),
        W2=np.asarray(inputs["W2"], np.float32),
        M0=M0, M1=M1, M2=M2,
        b0=np.asarray(inputs["b0"], np.float32).reshape(1, -1),
        b1=np.asarray(inputs["b1"], np.float32).reshape(1, -1),
        b2=np.asarray(inputs["b2"], np.float32).reshape(1, -1),
        ones=np.ones((1, 128), np.float32),
    )
    in_maps = []
    for c in range(n_cores):
        lay = st["layouts"][c]
        fo = np.zeros((npc, D_IN), np.float32)
        valid = lay >= 0
        fo[valid] = feats[lay[valid]]
        m = dict(shared)
        m["feats_own"] = fo
        m["idx_mega"] = st["idx_mega"][c]
        in_maps.append(m)
    return in_maps


def assemble_output(st, results, N, D_OUT=64):
    out = np.zeros((N, D_OUT), np.float32)
    for c, r in enumerate(results):
        lay = st["layouts"][c]
        valid = lay >= 0
        out[lay[valid]] = r["out"][valid]
    return out


# =================================================================== kernel

_CACHE = {}
LAST_EXEC_NS = None
LAST_TRACE = None
LAST_RES = None


def kernel(**inputs):
    """Full-input GAT forward on 8 NeuronCores; returns [50000, 64] f32."""
    global LAST_EXEC_NS
    from concourse.bass_utils import run_bass_kernel_spmd

    N = 50000
    src = np.asarray(inputs["src"])
    dst = np.asarray(inputs["dst"])

    if "prog" not in _CACHE:
        st = prep_host(src, dst, N, n_cores=8)
        nc = build_program(st)
        _CACHE["prog"] = (st, nc)
    st, nc = _CACHE["prog"]

    in_maps = make_in_maps(st, inputs)
    trace = os.environ.get("GAT_TRACE", "0") == "1"
    res = run_bass_kernel_spmd(nc, in_maps, core_ids=list(range(8)), trace=trace)
    LAST_EXEC_NS = res.exec_time_ns
    global LAST_TRACE, LAST_RES
    LAST_RES = res
    if res.instructions_and_trace:
        LAST_TRACE = res.instructions_and_trace[1]
    return assemble_output(st, [res.results[c] for c in range(8)], N)



# revision 9
# speedup vs baseline: 1.0914x; 1.0914x over previous
"""GAT (3-layer, 4/4/1 heads) on 8 TRN2 NeuronCores.

Design (per core, SPMD one program):
- Nodes dealt to cores by degree (snake); within a core, nodes are 2D-packed
  by (deg_A, deg_B) into 128-node blocks, blocks sorted by slot width desc.
- Records are fp16: [feat | el | pad] rows gathered per edge at 512B (L0/L1)
  / 256B (L2) granularity via SWDGE dma_gather (int16 idx -> table split in
  two contiguous halves; halves = chunk {0,1} vs {2,3} row ranges).
- Table rows are chunk-major ([chunk][core][rows]) so each chunk's AllGather
  output is contiguous; the 4 chunk AGs per layer overlap the previous edge
  phase (dense blocks for layer L+1 are interleaved into edge phase L).
- Edge phase per block: dma_gather src records, e = el_src + er_dst(bcast)
  -> leaky-relu -> rowmax-subtract -> exp (ACT) -> denom reduce -> msg =
  feat * ex (DVE fp16) -> slot-reduce fp32 -> scale+bias+relu.
- Softmax is exact (per-dst max subtracted), fp16 storage keeps rel err low.
"""

import os
import numpy as np

import concourse.bacc as bacc
import concourse.bass as bass
import concourse.tile as tile
from concourse import mybir
from concourse.masks import make_identity

F32 = mybir.dt.float32
F16 = mybir.dt.float16
I16 = mybir.dt.int16

PAD_EL = np.float16(-30000.0)
NCHUNK = 1


# ---------------------------------------------------------------- host prep

def prep_host(src, dst, N, n_cores=8, blk=128):
    """Graph -> per-core slot grids (owner-half A/B split) + AG chunks."""
    src = np.asarray(src); dst = np.asarray(dst)
    deg = np.bincount(dst, minlength=N)

    # snake-deal nodes by degree to balance per-core edge counts
    order = np.argsort(-deg, kind="stable")
    owner = np.empty(N, np.int32)
    for i, v in enumerate(order):
        r = i % (2 * n_cores)
        owner[v] = r if r < n_cores else 2 * n_cores - 1 - r

    half_of_core = (np.arange(n_cores) >= n_cores // 2).astype(np.int32)
    src_half = half_of_core[owner[src]]
    degA = np.bincount(dst[src_half == 0], minlength=N)
    degB = np.bincount(dst[src_half == 1], minlength=N)

    core_nodes = [np.where(owner == c)[0] for c in range(n_cores)]
    npc_real = max(len(cn) for cn in core_nodes)
    assert all(len(cn) == npc_real for cn in core_nodes)
    nblk = (npc_real + blk) // blk          # >=1 pad row per core
    npc = nblk * blk

    # chunk boundaries in blocks (AG pipelining)
    cb = [0]
    per = (nblk + NCHUNK - 1) // NCHUNK
    for k in range(NCHUNK):
        cb.append(min(nblk, cb[-1] + per))
    chunk_blocks = [(cb[k], cb[k + 1]) for k in range(NCHUNK)]

    # per-core: 2D pack by (degA, degB), blocks sorted by slot width desc
    layouts = []
    TA = np.zeros((n_cores, nblk), np.int32)
    TB = np.zeros((n_cores, nblk), np.int32)
    for c in range(n_cores):
        cn = core_nodes[c]
        key = np.maximum(degA[cn], degB[cn]) * 1000 + np.minimum(degA[cn], degB[cn])
        cn = cn[np.argsort(-key, kind="stable")]
        lay = np.full(npc, -1, np.int64)
        lay[:len(cn)] = cn
        for b in range(nblk):
            rowsb = lay[b * blk:(b + 1) * blk]
            real = rowsb[rowsb >= 0]
            TA[c, b] = degA[real].max() if len(real) else 0
            TB[c, b] = degB[real].max() if len(real) else 0
        bo = np.argsort(-(TA[c] + TB[c]), kind="stable")
        lay = np.concatenate([lay[b * blk:(b + 1) * blk] for b in bo])
        TA[c], TB[c] = TA[c][bo], TB[c][bo]
        layouts.append(lay)

    TAs = TA.max(axis=0)
    TBs = TB.max(axis=0)
    tot_T = int(TAs.sum() + TBs.sum())

    half_rows = (n_cores // 2) * npc
    assert half_rows < 32768
    pi = np.full(N, -1, np.int64)
    for c in range(n_cores):
        rowsb = layouts[c]
        valid = rowsb >= 0
        pi[rowsb[valid]] = c * npc + np.where(valid)[0]
    assert (pi[np.unique(dst)] >= 0).all()

    # in-edge lists per node
    eorder = np.argsort(dst, kind="stable")
    esrc = src[eorder]; edst = dst[eorder]
    estart = np.searchsorted(edst, np.arange(N))
    eend = np.searchsorted(edst, np.arange(N), side="right")

    # slot grids: per block columns [A: TAs(b) | B: TBs(b)]
    col_off = {}
    off = 0
    for b in range(nblk):
        col_off[(b, 0)] = (off, int(TAs[b])); off += int(TAs[b])
        col_off[(b, 1)] = (off, int(TBs[b])); off += int(TBs[b])
    assert off == tot_T

    idx_mega = np.zeros((n_cores, 128, 8 * tot_T), np.int16)
    for c in range(n_cores):
        lay = layouts[c]
        for b in range(nblk):
            rowsb = lay[b * blk:(b + 1) * blk]
            for half in (0, 1):
                o, T = col_off[(b, half)]
                if T == 0:
                    continue
                base = half * half_rows
                iv = np.full((128, T), half_rows - 1, np.int64)  # pad row
                for p in range(blk):
                    v = rowsb[p]
                    if v < 0:
                        continue
                    es = esrc[estart[v]:eend[v]]
                    r = pi[es]
                    mysrc = r[r >= half_rows] if half else r[r < half_rows]
                    k = len(mysrc)
                    assert k <= T
                    iv[p, :k] = mysrc - base
                assert (iv >= 0).all() and (iv < 32768).all()
                pos = np.arange(128 * T)
                wrapped = np.zeros((16, 8 * T), np.int16)
                wrapped[pos % 16, pos // 16] = iv[pos % 128, pos // 128].astype(np.int16)
                idx_mega[c, :, o * 8:(o + T) * 8] = np.tile(wrapped, (8, 1))

    return dict(
        n_cores=n_cores, blk=blk, nblk=nblk, npc=npc, n_real=npc_real,
        half_rows=half_rows, NROWS=n_cores * npc, tot_T=tot_T, col_off=col_off,
        TA=TAs, TB=TBs, layouts=layouts, pi=pi, chunk_blocks=chunk_blocks,
        idx_mega=idx_mega,
    )


# ------------------------------------------------------------ device build

def build_program(st, H=4, DH=32, D_IN=128, D_OUT=64, nq=4):
    n_cores, blk, nblk, npc = st["n_cores"], st["blk"], st["nblk"], st["npc"]
    tot_T, col_off = st["tot_T"], st["col_off"]
    HD = H * DH                      # 128
    REC = 256                        # fp16 elems per L0/L1 record (512B)
    REC2 = 128                       # fp16 elems per L2 record (256B)
    NROWS = st["NROWS"]
    half_A = st["half_rows"]
    chunk_blocks = st["chunk_blocks"]

    nc = bacc.Bacc(None, target_bir_lowering=False, num_swdge_queues=nq)

    feats_own = nc.dram_tensor("feats_own", [npc, D_IN], F32, kind="ExternalInput")
    idx_in = nc.dram_tensor("idx_mega", [128, 8 * tot_T], I16, kind="ExternalInput")
    W0_in = nc.dram_tensor("W0", [HD, D_IN], F32, kind="ExternalInput")
    W1_in = nc.dram_tensor("W1", [HD, HD], F32, kind="ExternalInput")
    W2_in = nc.dram_tensor("W2", [D_OUT, HD], F32, kind="ExternalInput")
    M0_in = nc.dram_tensor("M0", [HD, HD + 8], F32, kind="ExternalInput")
    M1_in = nc.dram_tensor("M1", [HD, HD + 8], F32, kind="ExternalInput")
    M2_in = nc.dram_tensor("M2", [D_OUT, D_OUT + 2], F32, kind="ExternalInput")
    b0_in = nc.dram_tensor("b0", [1, HD], F32, kind="ExternalInput")
    b1_in = nc.dram_tensor("b1", [1, HD], F32, kind="ExternalInput")
    b2_in = nc.dram_tensor("b2", [1, D_OUT], F32, kind="ExternalInput")
    ones_in = nc.dram_tensor("ones", [1, 128], F32, kind="ExternalInput")
    out_t = nc.dram_tensor("out", [npc, D_OUT], F32, kind="ExternalOutput")

    LCFG = [  # per layer: rec elems (fp16), feat width, heads, el col, wcat
        dict(rec=REC, fw=HD, h=H, elc=HD, wcat=HD + 8),
        dict(rec=REC, fw=HD, h=H, elc=HD, wcat=HD + 8),
        dict(rec=REC2, fw=D_OUT, h=1, elc=D_OUT, wcat=D_OUT + 2),
    ]

    with tile.TileContext(nc) as tc:
        with tc.tile_pool(name="const", bufs=1) as cpool, \
             tc.tile_pool(name="dram", bufs=1, space="DRAM") as dram, \
             tc.tile_pool(name="xblk", bufs=1) as xpool, \
             tc.tile_pool(name="gat", bufs=3) as gpool, \
             tc.tile_pool(name="msg", bufs=2) as mpool, \
             tc.tile_pool(name="small", bufs=4) as spool, \
             tc.tile_pool(name="psum", bufs=2, space="PSUM") as ppool, \
             tc.tile_pool(name="psum1", bufs=2, space="PSUM") as ppool2:

            # ---------------- constants / prologue
            idx_sb = cpool.tile([128, 8 * tot_T], I16)
            nc.sync.dma_start(out=idx_sb[:], in_=idx_in[:])
            ident = cpool.tile([128, 128], F32)
            make_identity(nc, ident[:])
            pad_el = cpool.tile([blk, 8], F16)
            nc.vector.memset(pad_el[:], float(PAD_EL))

            wins = [W0_in, W1_in, W2_in]
            mins = [M0_in, M1_in, M2_in]
            bins = [b0_in, b1_in, b2_in]
            wcat_sb, brep_sb = [], []
            with tc.tile_pool(name="prolog", bufs=1) as plpool:
                ones_sb = plpool.tile([1, 128], F32)
                nc.sync.dma_start(out=ones_sb[:], in_=ones_in[:])
                for L in range(3):
                    cfg = LCFG[L]
                    kdim = wins[L].shape[0]
                    w_sb = plpool.tile([kdim, wins[L].shape[1]], F32, tag=f"wld{L}", name=f"wld{L}")
                    nc.sync.dma_start(out=w_sb[:], in_=wins[L][:])
                    m_sb = plpool.tile([kdim, cfg["wcat"]], F32, tag=f"mld{L}", name=f"mld{L}")
                    nc.sync.dma_start(out=m_sb[:], in_=mins[L][:])
                    wc_ps = ppool.tile([wins[L].shape[1], cfg["wcat"]], F32, space="PSUM", tag="wcps")
                    nc.tensor.matmul(out=wc_ps[:], lhsT=w_sb[:], rhs=m_sb[:], start=True, stop=True)
                    wc = cpool.tile([wins[L].shape[1], cfg["wcat"]], F32, tag=f"wcat{L}", name=f"wcat{L}")
                    nc.vector.tensor_copy(out=wc[:], in_=wc_ps[:])
                    wcat_sb.append(wc)

                    b_sb = plpool.tile([1, cfg["fw"]], F32, tag=f"bld{L}", name=f"bld{L}")
                    nc.sync.dma_start(out=b_sb[:], in_=bins[L][:])
                    br_ps = ppool.tile([128, cfg["fw"]], F32, space="PSUM", tag="brps")
                    nc.tensor.matmul(out=br_ps[:], lhsT=ones_sb[:], rhs=b_sb[:], start=True, stop=True)
                    br = cpool.tile([128, cfg["fw"]], F32, tag=f"brep{L}", name=f"brep{L}")
                    nc.vector.tensor_copy(out=br[:], in_=br_ps[:])
                    brep_sb.append(br)

            # persistent per-block x tiles and er tiles
            x_tiles = [xpool.tile([128, HD], F32, tag=f"x{b}", name=f"xblk{b}") for b in range(nblk)]
            er_tiles = [xpool.tile([128, H], F16, tag=f"er{b}", name=f"erblk{b}") for b in range(nblk)]
            for b in range(nblk):
                if st["TA"][b] + st["TB"][b] == 0:
                    nc.vector.memset(x_tiles[b][:], 0)

            # record staging tiles (pad cols zeroed once)
            rec_tiles = [xpool.tile([128, REC], F16, tag=f"rec{i}", name=f"rect{i}") for i in range(2)]
            for i in range(2):
                nc.vector.memset(rec_tiles[i][:], 0)

            slabs = [dram.tile([npc, REC], F16, tag="slab0", name="slab0"),
                     dram.tile([npc, REC], F16, tag="slab1", name="slab1"),
                     dram.tile([npc, REC2], F16, tag="slab2", name="slab2")]
            tables = [dram.tile([NROWS, REC], F16, tag="tab0", name="tab0", addr_space="Shared"),
                      dram.tile([NROWS, REC], F16, tag="tab1", name="tab1", addr_space="Shared"),
                      dram.tile([NROWS, REC2], F16, tag="tab2", name="tab2", addr_space="Shared")]

            n_real = st["n_real"]

            def dense_block(L, b):
                """Record rows for owned block b of layer L (PE + copies)."""
                cfg = LCFG[L]
                if L == 0:
                    x_sb = spool.tile([128, D_IN], F32, tag="xts", name="x0ld")
                    nc.sync.dma_start(out=x_sb[:], in_=feats_own[b * blk:(b + 1) * blk, :])
                else:
                    x_sb = x_tiles[b]
                xt_ps = ppool.tile([128, 128], F32, space="PSUM", tag="xtps", name="xt_ps")
                nc.tensor.transpose(out=xt_ps[:], in_=x_sb[:], identity=ident[:])
                xt_sb = spool.tile([128, 128], F32, tag="xts", name="xt_sb")
                if b % 2 == 0:
                    nc.scalar.copy(out=xt_sb[:], in_=xt_ps[:])
                else:
                    nc.vector.tensor_copy(out=xt_sb[:], in_=xt_ps[:])
                y_ps = ppool2.tile([128, cfg["wcat"]], F32, space="PSUM", tag="yps", name="y_ps")
                nc.tensor.matmul(out=y_ps[:], lhsT=xt_sb[:], rhs=wcat_sb[L][:], start=True, stop=True)
                recb = rec_tiles[b % 2]
                # feat + el (er col dropped for L2; harmless extra er cols for L0/L1)
                ncopy = cfg["elc"] + cfg["h"]
                nc.scalar.copy(out=recb[:, 0:ncopy], in_=y_ps[:, 0:ncopy])
                nc.vector.tensor_copy(out=er_tiles[b][:, 0:cfg["h"]],
                                      in_=y_ps[:, cfg["elc"] + cfg["h"]:cfg["elc"] + 2 * cfg["h"]])
                nc.sync.dma_start(out=slabs[L][b * blk:(b + 1) * blk, :],
                                  in_=recb[:, 0:cfg["rec"]])

            def finish_chunk(L, k):
                """Pad-row el poison (last chunk) + AllGather chunk k."""
                cfg = LCFG[L]
                b0, b1 = chunk_blocks[k]
                if k == NCHUNK - 1:
                    nc.sync.dma_start(
                        out=slabs[L][n_real:npc, cfg["elc"]:cfg["elc"] + cfg["h"]],
                        in_=pad_el[0:npc - n_real, 0:cfg["h"]])
                if (b0, b1) == (0, nblk):
                    out_ap = tables[L][:].opt()
                else:
                    out_v = tables[L][:].rearrange("(c r) e -> c r e", c=n_cores)
                    out_ap = out_v[:, b0 * blk:b1 * blk, :]
                nc.gpsimd.collective_compute(
                    "AllGather", mybir.AluOpType.bypass,
                    replica_groups=[list(range(n_cores))],
                    ins=[slabs[L][b0 * blk:b1 * blk, :].opt()],
                    outs=[out_ap],
                )

            def chunk_of_block(b):
                for k, (b0, b1) in enumerate(chunk_blocks):
                    if b0 <= b < b1:
                        return k, b == b1 - 1
                raise AssertionError

            for b in range(nblk):
                dense_block(0, b)
                k, last = chunk_of_block(b)
                if last:
                    finish_chunk(0, k)

            for L in range(3):
                cfg = LCFG[L]
                h, fw, rec = cfg["h"], cfg["fw"], cfg["rec"]
                elc = cfg["elc"]
                dh = fw // h
                tabA = tables[L][0:half_A, :]
                tabB = tables[L][half_A:NROWS, :]

                gcount = 0
                for b in range(nblk):
                    oA, TA_ = col_off[(b, 0)]
                    oB, TB_ = col_off[(b, 1)]
                    T = TA_ + TB_
                    if T == 0:
                        continue
                    grid = gpool.tile([128, T, rec], F16, tag="grid")
                    for (hf, o, Th) in ((0, oA, TA_), (1, oB, TB_)):
                        goff = 0 if hf == 0 else TA_
                        for t0 in range(0, Th, 8):
                            tw = min(8, Th - t0)
                            nc.gpsimd.dma_gather(
                                out_ap=grid[:, goff + t0:goff + t0 + tw, :],
                                in_ap=(tabA if hf == 0 else tabB),
                                idxs_ap=idx_sb[:, (o + t0) * 8:(o + t0 + tw) * 8],
                                num_idxs=128 * tw,
                                num_idxs_reg=128 * tw,
                                elem_size=rec,
                                single_packet=True,
                                queue_num=(gcount % (2 * nq)) // 2,
                            )
                            gcount += 1

                    # e = el + er (bcast over slots), in place on el cols
                    el = grid[:, :, elc:elc + h]                  # [128,T,h] f16
                    erb = er_tiles[b][:, 0:h].unsqueeze(1).to_broadcast([128, T, h])
                    nc.vector.tensor_tensor(out=el, in0=el, in1=erb, op=mybir.AluOpType.add)
                    # leaky relu
                    nc.vector.scalar_tensor_tensor(
                        out=el, in0=el, scalar=0.2, in1=el,
                        op0=mybir.AluOpType.mult, op1=mybir.AluOpType.max)
                    # rowmax over slots, subtract (exact edge softmax)
                    mx = spool.tile([128, h], F16, tag="mx")
                    elT = el.rearrange("p t h -> p h t")
                    nc.vector.tensor_reduce(out=mx[:], in_=elT, axis=mybir.AxisListType.X,
                                            op=mybir.AluOpType.max)
                    mxb = mx[:, 0:h].unsqueeze(1).to_broadcast([128, T, h])
                    nc.vector.tensor_tensor(out=el, in0=el, in1=mxb, op=mybir.AluOpType.subtract)
                    # ex = exp(e - max)
                    nc.scalar.activation(out=el, in_=el, func=mybir.ActivationFunctionType.Exp)
                    # denominator + reciprocal
                    den = spool.tile([128, h], F32, tag="den")
                    nc.vector.tensor_reduce(out=den[:], in_=elT, axis=mybir.AxisListType.X,
                                            op=mybir.AluOpType.add)
                    rden = spool.tile([128, h], F32, tag="rden")
                    nc.vector.reciprocal(out=rden[:], in_=den[:])
                    # unnormalized msg = feat * ex
                    msg = mpool.tile([128, fw, T], F16, tag="msg")
                    feat_v = grid[:, :, 0:fw].rearrange("p t (h d) -> p h d t", h=h)
                    ex_v = el.rearrange("p t h -> p h t").unsqueeze(2).to_broadcast([128, h, dh, T])
                    msg_v = msg[:].rearrange("p (h d) t -> p h d t", h=h)
                    nc.vector.tensor_tensor(out=msg_v, in0=feat_v, in1=ex_v, op=mybir.AluOpType.mult)
                    # agg over slots (fp32 accumulate target)
                    agg = spool.tile([128, fw], F32, tag="agg")
                    nc.vector.tensor_reduce(out=agg[:], in_=msg[:],
                                            axis=mybir.AxisListType.X, op=mybir.AluOpType.add)
                    # x = (relu of) agg * rden + bias
                    rdb = rden[:, 0:h].unsqueeze(2).to_broadcast([128, h, dh])
                    agg_v = agg[:].rearrange("p (h d) -> p h d", h=h)
                    nc.vector.tensor_tensor(out=agg_v, in0=agg_v, in1=rdb, op=mybir.AluOpType.mult)
                    if L < 2:
                        nc.vector.tensor_tensor(out=agg[:], in0=agg[:], in1=brep_sb[L][:],
                                                op=mybir.AluOpType.add)
                        nc.scalar.activation(out=x_tiles[b][:], in_=agg[:],
                                             func=mybir.ActivationFunctionType.Relu)
                        dense_block(L + 1, b)
                        k, lastb = chunk_of_block(b)
                        if lastb:
                            finish_chunk(L + 1, k)
                    else:
                        ob = spool.tile([128, D_OUT], F32, tag="ob")
                        nc.vector.tensor_tensor(out=ob[:], in0=agg[:], in1=brep_sb[2][:],
                                                op=mybir.AluOpType.add)
                        nc.sync.dma_start(out=out_t[b * blk:(b + 1) * blk, :], in_=ob[:])

                if L < 2:
                    for b in range(nblk):
                        if st["TA"][b] + st["TB"][b] == 0:
                            dense_block(L + 1, b)
                            k, lastb = chunk_of_block(b)
                            if lastb:
                                finish_chunk(L + 1, k)

    nc.compile()
    return nc


# ------------------------------------------------------------------- runner

def make_in_maps(st, inputs, H=4, DH=32, D_IN=128, D_OUT=64):
    n_cores, npc = st["n_cores"], st["npc"]
    feats = np.asarray(inputs["feats"], np.float32)
    HD = H * DH

    def mk_M(al, ar, d):
        h = al.shape[0]
        M = np.zeros((d, d + 2 * h), np.float32)
        M[:, :d] = np.eye(d, dtype=np.float32)
        for hh in range(h):
            M[hh * (d // h):(hh + 1) * (d // h), d + hh] = al[hh]
            M[hh * (d // h):(hh + 1) * (d // h), d + h + hh] = ar[hh]
        return M

    M0 = mk_M(np.asarray(inputs["al0"]), np.asarray(inputs["ar0"]), HD)
    M1 = mk_M(np.asarray(inputs["al1"]), np.asarray(inputs["ar1"]), HD)
    M2 = mk_M(np.asarray(inputs["al2"]), np.asarray(inputs["ar2"]), D_OUT)

    shared = dict(
        W0=np.asarray(inputs["W0"], np.float32),
        W1=np.asarray(inputs["W1"], np.float32),
        W2=np.asarray(inputs["W2"], np.float32),
        M0=M0, M1=M1, M2=M2,
        b0=np.asarray(inputs["b0"], np.float32).reshape(1, -1),
        b1=np.asarray(inputs["b1"], np.float32).reshape(1, -1),
        b2=np.asarray(inputs["b2"], np.float32).reshape(1, -1),
        ones=np.ones((1, 128), np.float32),
    )
    in_maps = []
    for c in range(n_cores):
        lay = st["layouts"][c]
        fo = np.zeros((npc, D_IN), np.float32)
        valid = lay >= 0
        fo[valid] = feats[lay[valid]]
        m = dict(shared)
        m["feats_own"] = fo
        m["idx_mega"] = st["idx_mega"][c]
        in_maps.append(m)
    return in_maps


def assemble_output(st, results, N, D_OUT=64):
    out = np.zeros((N, D_OUT), np.float32)
    for c, r in enumerate(results):
        lay = st["layouts"][c]
        valid = lay >= 0
        out[lay[valid]] = r["out"][valid]
    return out


# =================================================================== kernel

_CACHE = {}
LAST_EXEC_NS = None
LAST_TRACE = None
LAST_RES = None


def kernel(**inputs):
    """Full-input GAT forward on 8 NeuronCores; returns [50000, 64] f32."""
    global LAST_EXEC_NS, LAST_TRACE, LAST_RES
    from concourse.bass_utils import run_bass_kernel_spmd

    N = 50000
    src = np.asarray(inputs["src"])
    dst = np.asarray(inputs["dst"])

    if "prog" not in _CACHE:
        st = prep_host(src, dst, N, n_cores=8)
        nc = build_program(st)
        _CACHE["prog"] = (st, nc)
    st, nc = _CACHE["prog"]

    in_maps = make_in_maps(st, inputs)
    trace = os.environ.get("GAT_TRACE", "0") == "1"
    res = run_bass_kernel_spmd(nc, in_maps, core_ids=list(range(8)), trace=trace)
    LAST_EXEC_NS = res.exec_time_ns
    LAST_RES = res
    if res.instructions_and_trace:
        LAST_TRACE = res.instructions_and_trace[1]
    return assemble_output(st, [res.results[c] for c in range(8)], N)


# revision 13
# speedup vs baseline: 1.1873x; 1.0879x over previous
"""GAT (3-layer, 4/4/1 heads) on 8 TRN2 NeuronCores.

Design (per core, SPMD one program):
- Nodes dealt to cores by degree (snake); within a core, nodes are 2D-packed
  by (deg_A, deg_B) into 128-node blocks, blocks sorted by slot width desc.
- Records are fp16: [feat | el | pad] rows gathered per edge at 512B (L0/L1)
  / 256B (L2) granularity via SWDGE dma_gather (int16 idx -> table split in
  two contiguous halves; halves = chunk {0,1} vs {2,3} row ranges).
- Table rows are chunk-major ([chunk][core][rows]) so each chunk's AllGather
  output is contiguous; the 4 chunk AGs per layer overlap the previous edge
  phase (dense blocks for layer L+1 are interleaved into edge phase L).
- Edge phase per block: dma_gather src records, e = el_src + er_dst(bcast)
  -> leaky-relu -> rowmax-subtract -> exp (ACT) -> denom reduce -> msg =
  feat * ex (DVE fp16) -> slot-reduce fp32 -> scale+bias+relu.
- Softmax is exact (per-dst max subtracted), fp16 storage keeps rel err low.
"""

import os
import numpy as np

import concourse.bacc as bacc
import concourse.bass as bass
import concourse.tile as tile
from concourse import mybir
from concourse.masks import make_identity

F32 = mybir.dt.float32
F16 = mybir.dt.float16
I16 = mybir.dt.int16

PAD_EL = np.float16(-30000.0)
NCHUNK = 1


# ---------------------------------------------------------------- host prep

def prep_host(src, dst, N, n_cores=8, blk=128):
    """Graph -> per-core slot grids (owner-half A/B split) + AG chunks."""
    src = np.asarray(src); dst = np.asarray(dst)
    deg = np.bincount(dst, minlength=N)

    # snake-deal nodes by degree to balance per-core edge counts
    order = np.argsort(-deg, kind="stable")
    owner = np.empty(N, np.int32)
    for i, v in enumerate(order):
        r = i % (2 * n_cores)
        owner[v] = r if r < n_cores else 2 * n_cores - 1 - r

    half_of_core = (np.arange(n_cores) >= n_cores // 2).astype(np.int32)
    src_half = half_of_core[owner[src]]
    degA = np.bincount(dst[src_half == 0], minlength=N)
    degB = np.bincount(dst[src_half == 1], minlength=N)

    core_nodes = [np.where(owner == c)[0] for c in range(n_cores)]
    npc_real = max(len(cn) for cn in core_nodes)
    assert all(len(cn) == npc_real for cn in core_nodes)
    nblk = (npc_real + blk) // blk          # >=1 pad row per core
    npc = nblk * blk

    # chunk boundaries in blocks (AG pipelining)
    cb = [0]
    per = (nblk + NCHUNK - 1) // NCHUNK
    for k in range(NCHUNK):
        cb.append(min(nblk, cb[-1] + per))
    chunk_blocks = [(cb[k], cb[k + 1]) for k in range(NCHUNK)]

    # per-core: 2D pack by (degA, degB), blocks sorted by slot width desc
    layouts = []
    TA = np.zeros((n_cores, nblk), np.int32)
    TB = np.zeros((n_cores, nblk), np.int32)
    for c in range(n_cores):
        cn = core_nodes[c]
        key = np.maximum(degA[cn], degB[cn]) * 1000 + np.minimum(degA[cn], degB[cn])
        cn = cn[np.argsort(-key, kind="stable")]
        lay = np.full(npc, -1, np.int64)
        lay[:len(cn)] = cn
        for b in range(nblk):
            rowsb = lay[b * blk:(b + 1) * blk]
            real = rowsb[rowsb >= 0]
            TA[c, b] = degA[real].max() if len(real) else 0
            TB[c, b] = degB[real].max() if len(real) else 0
        bo = np.argsort(-(TA[c] + TB[c]), kind="stable")
        lay = np.concatenate([lay[b * blk:(b + 1) * blk] for b in bo])
        TA[c], TB[c] = TA[c][bo], TB[c][bo]
        layouts.append(lay)

    TAs = TA.max(axis=0)
    TBs = TB.max(axis=0)
    tot_T = int(TAs.sum() + TBs.sum())

    half_rows = (n_cores // 2) * npc
    assert half_rows < 32768
    pi = np.full(N, -1, np.int64)
    for c in range(n_cores):
        rowsb = layouts[c]
        valid = rowsb >= 0
        pi[rowsb[valid]] = c * npc + np.where(valid)[0]
    assert (pi[np.unique(dst)] >= 0).all()

    # in-edge lists per node
    eorder = np.argsort(dst, kind="stable")
    esrc = src[eorder]; edst = dst[eorder]
    estart = np.searchsorted(edst, np.arange(N))
    eend = np.searchsorted(edst, np.arange(N), side="right")

    # slot grids: per block columns [A: TAs(b) | B: TBs(b)]
    col_off = {}
    off = 0
    for b in range(nblk):
        col_off[(b, 0)] = (off, int(TAs[b])); off += int(TAs[b])
        col_off[(b, 1)] = (off, int(TBs[b])); off += int(TBs[b])
    assert off == tot_T

    idx_mega = np.zeros((n_cores, 128, 8 * tot_T), np.int16)
    for c in range(n_cores):
        lay = layouts[c]
        for b in range(nblk):
            rowsb = lay[b * blk:(b + 1) * blk]
            for half in (0, 1):
                o, T = col_off[(b, half)]
                if T == 0:
                    continue
                base = half * half_rows
                iv = np.full((128, T), half_rows - 1, np.int64)  # pad row
                for p in range(blk):
                    v = rowsb[p]
                    if v < 0:
                        continue
                    es = esrc[estart[v]:eend[v]]
                    r = pi[es]
                    mysrc = r[r >= half_rows] if half else r[r < half_rows]
                    k = len(mysrc)
                    assert k <= T
                    iv[p, :k] = mysrc - base
                assert (iv >= 0).all() and (iv < 32768).all()
                pos = np.arange(128 * T)
                wrapped = np.zeros((16, 8 * T), np.int16)
                wrapped[pos % 16, pos // 16] = iv[pos % 128, pos // 128].astype(np.int16)
                idx_mega[c, :, o * 8:(o + T) * 8] = np.tile(wrapped, (8, 1))

    return dict(
        n_cores=n_cores, blk=blk, nblk=nblk, npc=npc, n_real=npc_real,
        half_rows=half_rows, NROWS=n_cores * npc, tot_T=tot_T, col_off=col_off,
        TA=TAs, TB=TBs, layouts=layouts, pi=pi, chunk_blocks=chunk_blocks,
        idx_mega=idx_mega,
    )


# ------------------------------------------------------------ device build

def build_program(st, H=4, DH=32, D_IN=128, D_OUT=64, nq=4):
    n_cores, blk, nblk, npc = st["n_cores"], st["blk"], st["nblk"], st["npc"]
    tot_T, col_off = st["tot_T"], st["col_off"]
    HD = H * DH                      # 128
    REC = 256                        # fp16 elems per L0/L1 record (512B)
    REC2 = 128                       # fp16 elems per L2 record (256B)
    NROWS = st["NROWS"]
    half_A = st["half_rows"]
    chunk_blocks = st["chunk_blocks"]

    nc = bacc.Bacc(None, target_bir_lowering=False, num_swdge_queues=nq)

    feats_own = nc.dram_tensor("feats_own", [npc, D_IN], F32, kind="ExternalInput")
    idx_in = nc.dram_tensor("idx_mega", [128, 8 * tot_T], I16, kind="ExternalInput")
    W0_in = nc.dram_tensor("W0", [HD, D_IN], F32, kind="ExternalInput")
    W1_in = nc.dram_tensor("W1", [HD, HD], F32, kind="ExternalInput")
    W2_in = nc.dram_tensor("W2", [D_OUT, HD], F32, kind="ExternalInput")
    M0_in = nc.dram_tensor("M0", [HD, HD + 8], F32, kind="ExternalInput")
    M1_in = nc.dram_tensor("M1", [HD, HD + 8], F32, kind="ExternalInput")
    M2_in = nc.dram_tensor("M2", [D_OUT, D_OUT + 2], F32, kind="ExternalInput")
    b0_in = nc.dram_tensor("b0", [1, HD], F32, kind="ExternalInput")
    b1_in = nc.dram_tensor("b1", [1, HD], F32, kind="ExternalInput")
    b2_in = nc.dram_tensor("b2", [1, D_OUT], F32, kind="ExternalInput")
    ones_in = nc.dram_tensor("ones", [1, 128], F32, kind="ExternalInput")
    out_t = nc.dram_tensor("out", [npc, D_OUT], F32, kind="ExternalOutput")

    LCFG = [  # per layer: rec elems (fp16), feat width, heads, el col, wcat
        dict(rec=REC, fw=HD, h=H, elc=HD, wcat=HD + 8),
        dict(rec=REC, fw=HD, h=H, elc=HD, wcat=HD + 8),
        dict(rec=REC2, fw=D_OUT, h=1, elc=D_OUT, wcat=D_OUT + 2),
    ]

    with tile.TileContext(nc) as tc:
        with tc.tile_pool(name="const", bufs=1) as cpool, \
             tc.tile_pool(name="dram", bufs=1, space="DRAM") as dram, \
             tc.tile_pool(name="xblk", bufs=1) as xpool, \
             tc.tile_pool(name="gat", bufs=4) as gpool, \
             tc.tile_pool(name="small", bufs=4) as spool, \
             tc.tile_pool(name="psum", bufs=2, space="PSUM") as ppool, \
             tc.tile_pool(name="psum1", bufs=2, space="PSUM") as ppool2:

            # ---------------- constants / prologue
            idx_sb = cpool.tile([128, 8 * tot_T], I16)
            nc.sync.dma_start(out=idx_sb[:], in_=idx_in[:])
            ident = cpool.tile([128, 128], F32)
            make_identity(nc, ident[:])
            pad_el = cpool.tile([blk, 8], F16)
            nc.vector.memset(pad_el[:], float(PAD_EL))

            wins = [W0_in, W1_in, W2_in]
            mins = [M0_in, M1_in, M2_in]
            bins = [b0_in, b1_in, b2_in]
            wcat_sb, brep_sb = [], []
            with tc.tile_pool(name="prolog", bufs=1) as plpool:
                ones_sb = plpool.tile([1, 128], F32)
                nc.sync.dma_start(out=ones_sb[:], in_=ones_in[:])
                for L in range(3):
                    cfg = LCFG[L]
                    kdim = wins[L].shape[0]
                    w_sb = plpool.tile([kdim, wins[L].shape[1]], F32, tag=f"wld{L}", name=f"wld{L}")
                    nc.sync.dma_start(out=w_sb[:], in_=wins[L][:])
                    m_sb = plpool.tile([kdim, cfg["wcat"]], F32, tag=f"mld{L}", name=f"mld{L}")
                    nc.sync.dma_start(out=m_sb[:], in_=mins[L][:])
                    wc_ps = ppool.tile([wins[L].shape[1], cfg["wcat"]], F32, space="PSUM", tag="wcps")
                    nc.tensor.matmul(out=wc_ps[:], lhsT=w_sb[:], rhs=m_sb[:], start=True, stop=True)
                    wc = cpool.tile([wins[L].shape[1], cfg["wcat"]], F32, tag=f"wcat{L}", name=f"wcat{L}")
                    nc.vector.tensor_copy(out=wc[:], in_=wc_ps[:])
                    wcat_sb.append(wc)

                    b_sb = plpool.tile([1, cfg["fw"]], F32, tag=f"bld{L}", name=f"bld{L}")
                    nc.sync.dma_start(out=b_sb[:], in_=bins[L][:])
                    br_ps = ppool.tile([128, cfg["fw"]], F32, space="PSUM", tag="brps")
                    nc.tensor.matmul(out=br_ps[:], lhsT=ones_sb[:], rhs=b_sb[:], start=True, stop=True)
                    br = cpool.tile([128, cfg["fw"]], F32, tag=f"brep{L}", name=f"brep{L}")
                    nc.vector.tensor_copy(out=br[:], in_=br_ps[:])
                    brep_sb.append(br)

            # persistent per-block x tiles and er tiles
            x_tiles = [xpool.tile([128, HD], F32, tag=f"x{b}", name=f"xblk{b}") for b in range(nblk)]
            er_tiles = [xpool.tile([128, H], F16, tag=f"er{b}", name=f"erblk{b}") for b in range(nblk)]
            for b in range(nblk):
                if st["TA"][b] + st["TB"][b] == 0:
                    nc.vector.memset(x_tiles[b][:], 0)

            # record staging tiles (pad cols zeroed once)
            rec_tiles = [xpool.tile([128, REC], F16, tag=f"rec{i}", name=f"rect{i}") for i in range(2)]
            for i in range(2):
                nc.vector.memset(rec_tiles[i][:], 0)

            slabs = [dram.tile([npc, REC], F16, tag="slab0", name="slab0"),
                     dram.tile([npc, REC], F16, tag="slab1", name="slab1"),
                     dram.tile([npc, REC2], F16, tag="slab2", name="slab2")]
            tables = [dram.tile([NROWS, REC], F16, tag="tab0", name="tab0", addr_space="Shared"),
                      dram.tile([NROWS, REC], F16, tag="tab1", name="tab1", addr_space="Shared"),
                      dram.tile([NROWS, REC2], F16, tag="tab2", name="tab2", addr_space="Shared")]

            n_real = st["n_real"]

            def dense_block(L, b):
                """Record rows for owned block b of layer L (PE + copies)."""
                cfg = LCFG[L]
                if L == 0:
                    x_sb = spool.tile([128, D_IN], F32, tag="xts", name="x0ld")
                    nc.sync.dma_start(out=x_sb[:], in_=feats_own[b * blk:(b + 1) * blk, :])
                else:
                    x_sb = x_tiles[b]
                xt_ps = ppool.tile([128, 128], F32, space="PSUM", tag="xtps", name="xt_ps")
                nc.tensor.transpose(out=xt_ps[:], in_=x_sb[:], identity=ident[:])
                xt_sb = spool.tile([128, 128], F32, tag="xts", name="xt_sb")
                if b % 2 == 0:
                    nc.scalar.copy(out=xt_sb[:], in_=xt_ps[:])
                else:
                    nc.vector.tensor_copy(out=xt_sb[:], in_=xt_ps[:])
                y_ps = ppool2.tile([128, cfg["wcat"]], F32, space="PSUM", tag="yps", name="y_ps")
                nc.tensor.matmul(out=y_ps[:], lhsT=xt_sb[:], rhs=wcat_sb[L][:], start=True, stop=True)
                recb = rec_tiles[b % 2]
                # feat + el (er col dropped for L2; harmless extra er cols for L0/L1)
                ncopy = cfg["elc"] + cfg["h"]
                nc.scalar.copy(out=recb[:, 0:ncopy], in_=y_ps[:, 0:ncopy])
                nc.vector.tensor_copy(out=er_tiles[b][:, 0:cfg["h"]],
                                      in_=y_ps[:, cfg["elc"] + cfg["h"]:cfg["elc"] + 2 * cfg["h"]])
                nc.sync.dma_start(out=slabs[L][b * blk:(b + 1) * blk, :],
                                  in_=recb[:, 0:cfg["rec"]])

            def finish_chunk(L, k):
                """Pad-row el poison (last chunk) + AllGather chunk k."""
                cfg = LCFG[L]
                b0, b1 = chunk_blocks[k]
                if k == NCHUNK - 1:
                    nc.sync.dma_start(
                        out=slabs[L][n_real:npc, cfg["elc"]:cfg["elc"] + cfg["h"]],
                        in_=pad_el[0:npc - n_real, 0:cfg["h"]])
                if (b0, b1) == (0, nblk):
                    out_ap = tables[L][:].opt()
                else:
                    out_v = tables[L][:].rearrange("(c r) e -> c r e", c=n_cores)
                    out_ap = out_v[:, b0 * blk:b1 * blk, :]
                nc.gpsimd.collective_compute(
                    "AllGather", mybir.AluOpType.bypass,
                    replica_groups=[list(range(n_cores))],
                    ins=[slabs[L][b0 * blk:b1 * blk, :].opt()],
                    outs=[out_ap],
                )

            def chunk_of_block(b):
                for k, (b0, b1) in enumerate(chunk_blocks):
                    if b0 <= b < b1:
                        return k, b == b1 - 1
                raise AssertionError

            for b in range(nblk):
                dense_block(0, b)
                k, last = chunk_of_block(b)
                if last:
                    finish_chunk(0, k)

            for L in range(3):
                cfg = LCFG[L]
                h, fw, rec = cfg["h"], cfg["fw"], cfg["rec"]
                elc = cfg["elc"]
                dh = fw // h
                tabA = tables[L][0:half_A, :]
                tabB = tables[L][half_A:NROWS, :]

                gcount = 0
                for b in range(nblk):
                    oA, TA_ = col_off[(b, 0)]
                    oB, TB_ = col_off[(b, 1)]
                    T = TA_ + TB_
                    if T == 0:
                        continue
                    grid = gpool.tile([128, T, rec], F16, tag="grid")
                    for (hf, o, Th) in ((0, oA, TA_), (1, oB, TB_)):
                        goff = 0 if hf == 0 else TA_
                        for t0 in range(0, Th, 16):
                            tw = min(16, Th - t0)
                            nc.gpsimd.dma_gather(
                                out_ap=grid[:, goff + t0:goff + t0 + tw, :],
                                in_ap=(tabA if hf == 0 else tabB),
                                idxs_ap=idx_sb[:, (o + t0) * 8:(o + t0 + tw) * 8],
                                num_idxs=128 * tw,
                                num_idxs_reg=128 * tw,
                                elem_size=rec,
                                single_packet=(tw <= 8),
                                queue_num=gcount % nq,
                            )
                            gcount += 1

                    # e = el + er (bcast over slots), in place on el cols
                    el = grid[:, :, elc:elc + h]                  # [128,T,h] f16
                    erb = er_tiles[b][:, 0:h].unsqueeze(1).to_broadcast([128, T, h])
                    nc.vector.tensor_tensor(out=el, in0=el, in1=erb, op=mybir.AluOpType.add)
                    # leaky relu
                    nc.vector.scalar_tensor_tensor(
                        out=el, in0=el, scalar=0.2, in1=el,
                        op0=mybir.AluOpType.mult, op1=mybir.AluOpType.max)
                    # rowmax over slots, subtract (exact edge softmax)
                    mx = spool.tile([128, h], F16, tag="mx")
                    elT = el.rearrange("p t h -> p h t")
                    nc.vector.tensor_reduce(out=mx[:], in_=elT, axis=mybir.AxisListType.X,
                                            op=mybir.AluOpType.max)
                    mxb = mx[:, 0:h].unsqueeze(1).to_broadcast([128, T, h])
                    nc.vector.tensor_tensor(out=el, in0=el, in1=mxb, op=mybir.AluOpType.subtract)
                    # ex = exp(e - max)
                    nc.scalar.activation(out=el, in_=el, func=mybir.ActivationFunctionType.Exp)
                    # denominator + reciprocal
                    den = spool.tile([128, h], F32, tag="den")
                    nc.vector.tensor_reduce(out=den[:], in_=elT, axis=mybir.AxisListType.X,
                                            op=mybir.AluOpType.add)
                    rden = spool.tile([128, h], F32, tag="rden")
                    nc.vector.reciprocal(out=rden[:], in_=den[:])
                    # unnormalized msg = feat * ex, in place on the grid so
                    # every AP walks contiguous 256B runs (strided views
                    # halve DVE throughput)
                    feat3 = grid[:, :, 0:fw].rearrange("p t (h d) -> p t h d", h=h)
                    exb = el.unsqueeze(3).to_broadcast([128, T, h, dh])
                    nc.vector.tensor_tensor(out=feat3, in0=feat3, in1=exb, op=mybir.AluOpType.mult)
                    # slot reduction as a binary tree of contiguous adds
                    Tr = T
                    while Tr > 1:
                        hlf = Tr // 2
                        dsts = grid[:, 0:hlf, 0:fw]
                        srcs = grid[:, Tr - hlf:Tr, 0:fw]
                        nc.vector.tensor_tensor(out=dsts, in0=dsts, in1=srcs, op=mybir.AluOpType.add)
                        Tr -= hlf
                    agg = spool.tile([128, fw], F32, tag="agg")
                    nc.vector.tensor_copy(out=agg[:], in_=grid[:, 0, 0:fw])
                    # x = (relu of) agg * rden + bias
                    rdb = rden[:, 0:h].unsqueeze(2).to_broadcast([128, h, dh])
                    agg_v = agg[:].rearrange("p (h d) -> p h d", h=h)
                    nc.vector.tensor_tensor(out=agg_v, in0=agg_v, in1=rdb, op=mybir.AluOpType.mult)
                    if L < 2:
                        nc.vector.tensor_tensor(out=agg[:], in0=agg[:], in1=brep_sb[L][:],
                                                op=mybir.AluOpType.add)
                        nc.scalar.activation(out=x_tiles[b][:], in_=agg[:],
                                             func=mybir.ActivationFunctionType.Relu)
                        dense_block(L + 1, b)
                        k, lastb = chunk_of_block(b)
                        if lastb:
                            finish_chunk(L + 1, k)
                    else:
                        ob = spool.tile([128, D_OUT], F32, tag="ob")
                        nc.vector.tensor_tensor(out=ob[:], in0=agg[:], in1=brep_sb[2][:],
                                                op=mybir.AluOpType.add)
                        nc.sync.dma_start(out=out_t[b * blk:(b + 1) * blk, :], in_=ob[:])

                if L < 2:
                    for b in range(nblk):
                        if st["TA"][b] + st["TB"][b] == 0:
                            dense_block(L + 1, b)
                            k, lastb = chunk_of_block(b)
                            if lastb:
                                finish_chunk(L + 1, k)

    nc.compile()
    return nc


# ------------------------------------------------------------------- runner

def make_in_maps(st, inputs, H=4, DH=32, D_IN=128, D_OUT=64):
    n_cores, npc = st["n_cores"], st["npc"]
    feats = np.asarray(inputs["feats"], np.float32)
    HD = H * DH

    def mk_M(al, ar, d):
        h = al.shape[0]
        M = np.zeros((d, d + 2 * h), np.float32)
        M[:, :d] = np.eye(d, dtype=np.float32)
        for hh in range(h):
            M[hh * (d // h):(hh + 1) * (d // h), d + hh] = al[hh]
            M[hh * (d // h):(hh + 1) * (d // h), d + h + hh] = ar[hh]
        return M

    M0 = mk_M(np.asarray(inputs["al0"]), np.asarray(inputs["ar0"]), HD)
    M1 = mk_M(np.asarray(inputs["al1"]), np.asarray(inputs["ar1"]), HD)
    M2 = mk_M(np.asarray(inputs["al2"]), np.asarray(inputs["ar2"]), D_OUT)

    shared = dict(
        W0=np.asarray(inputs["W0"], np.float32),
        W1=np.asarray(inputs["W1"], np.float32),
        W2=np.asarray(inputs["W2"], np.float32),
        M0=M0, M1=M1, M2=M2,
        b0=np.asarray(inputs["b0"], np.float32).reshape(1, -1),
        b1=np.asarray(inputs["b1"], np.float32).reshape(1, -1),
        b2=np.asarray(inputs["b2"], np.float32).reshape(1, -1),
        ones=np.ones((1, 128), np.float32),
    )
    in_maps = []
    for c in range(n_cores):
        lay = st["layouts"][c]
        fo = np.zeros((npc, D_IN), np.float32)
        valid = lay >= 0
        fo[valid] = feats[lay[valid]]
        m = dict(shared)
        m["feats_own"] = fo
        m["idx_mega"] = st["idx_mega"][c]
        in_maps.append(m)
    return in_maps


def assemble_output(st, results, N, D_OUT=64):
    out = np.zeros((N, D_OUT), np.float32)
    for c, r in enumerate(results):
        lay = st["layouts"][c]
        valid = lay >= 0
        out[lay[valid]] = r["out"][valid]
    return out


# =================================================================== kernel

_CACHE = {}
LAST_EXEC_NS = None
LAST_TRACE = None
LAST_RES = None


def kernel(**inputs):
    """Full-input GAT forward on 8 NeuronCores; returns [50000, 64] f32."""
    global LAST_EXEC_NS, LAST_TRACE, LAST_RES
    from concourse.bass_utils import run_bass_kernel_spmd

    N = 50000
    src = np.asarray(inputs["src"])
    dst = np.asarray(inputs["dst"])

    if "prog" not in _CACHE:
        st = prep_host(src, dst, N, n_cores=8)
        nc = build_program(st)
        _CACHE["prog"] = (st, nc)
    st, nc = _CACHE["prog"]

    in_maps = make_in_maps(st, inputs)
    trace = os.environ.get("GAT_TRACE", "0") == "1"
    res = run_bass_kernel_spmd(nc, in_maps, core_ids=list(range(8)), trace=trace)
    LAST_EXEC_NS = res.exec_time_ns
    LAST_RES = res
    if res.instructions_and_trace:
        LAST_TRACE = res.instructions_and_trace[1]
    return assemble_output(st, [res.results[c] for c in range(8)], N)


# revision 14
# speedup vs baseline: 1.3054x; 1.0995x over previous
"""GAT (3-layer, 4/4/1 heads) on 8 TRN2 NeuronCores.

Design (per core, SPMD one program):
- Nodes dealt to cores by degree (snake); within a core, nodes are 2D-packed
  by (deg_A, deg_B) into 128-node blocks, blocks sorted by slot width desc.
- Records are fp16: [feat | el | pad] rows gathered per edge at 512B (L0/L1)
  / 256B (L2) granularity via SWDGE dma_gather (int16 idx -> table split in
  two contiguous halves; halves = chunk {0,1} vs {2,3} row ranges).
- Table rows are chunk-major ([chunk][core][rows]) so each chunk's AllGather
  output is contiguous; the 4 chunk AGs per layer overlap the previous edge
  phase (dense blocks for layer L+1 are interleaved into edge phase L).
- Edge phase per block: dma_gather src records, e = el_src + er_dst(bcast)
  -> leaky-relu -> rowmax-subtract -> exp (ACT) -> denom reduce -> msg =
  feat * ex (DVE fp16) -> slot-reduce fp32 -> scale+bias+relu.
- Softmax is exact (per-dst max subtracted), fp16 storage keeps rel err low.
"""

import os
import numpy as np

import concourse.bacc as bacc
import concourse.bass as bass
import concourse.tile as tile
from concourse import mybir
from concourse.masks import make_identity

F32 = mybir.dt.float32
F16 = mybir.dt.float16
I16 = mybir.dt.int16

PAD_EL = np.float16(-30000.0)
NCHUNK = 1


# ---------------------------------------------------------------- host prep

def prep_host(src, dst, N, n_cores=8, blk=128):
    """Graph -> per-core slot grids (owner-half A/B split) + AG chunks."""
    src = np.asarray(src); dst = np.asarray(dst)
    deg = np.bincount(dst, minlength=N)

    # snake-deal nodes by degree to balance per-core edge counts
    order = np.argsort(-deg, kind="stable")
    owner = np.empty(N, np.int32)
    for i, v in enumerate(order):
        r = i % (2 * n_cores)
        owner[v] = r if r < n_cores else 2 * n_cores - 1 - r

    half_of_core = (np.arange(n_cores) >= n_cores // 2).astype(np.int32)
    src_half = half_of_core[owner[src]]
    degA = np.bincount(dst[src_half == 0], minlength=N)
    degB = np.bincount(dst[src_half == 1], minlength=N)

    core_nodes = [np.where(owner == c)[0] for c in range(n_cores)]
    npc_real = max(len(cn) for cn in core_nodes)
    assert all(len(cn) == npc_real for cn in core_nodes)
    nblk = (npc_real + blk) // blk          # >=1 pad row per core
    npc = nblk * blk

    # chunk boundaries in blocks (AG pipelining)
    cb = [0]
    per = (nblk + NCHUNK - 1) // NCHUNK
    for k in range(NCHUNK):
        cb.append(min(nblk, cb[-1] + per))
    chunk_blocks = [(cb[k], cb[k + 1]) for k in range(NCHUNK)]

    # per-core: 2D pack by (degA, degB), blocks sorted by slot width desc
    layouts = []
    TA = np.zeros((n_cores, nblk), np.int32)
    TB = np.zeros((n_cores, nblk), np.int32)
    for c in range(n_cores):
        cn = core_nodes[c]
        key = np.maximum(degA[cn], degB[cn]) * 1000 + np.minimum(degA[cn], degB[cn])
        cn = cn[np.argsort(-key, kind="stable")]
        lay = np.full(npc, -1, np.int64)
        lay[:len(cn)] = cn
        for b in range(nblk):
            rowsb = lay[b * blk:(b + 1) * blk]
            real = rowsb[rowsb >= 0]
            TA[c, b] = degA[real].max() if len(real) else 0
            TB[c, b] = degB[real].max() if len(real) else 0
        bo = np.argsort(-(TA[c] + TB[c]), kind="stable")
        lay = np.concatenate([lay[b * blk:(b + 1) * blk] for b in bo])
        TA[c], TB[c] = TA[c][bo], TB[c][bo]
        layouts.append(lay)

    TAs = TA.max(axis=0)
    TBs = TB.max(axis=0)
    tot_T = int(TAs.sum() + TBs.sum())

    half_rows = (n_cores // 2) * npc
    assert half_rows < 32768
    pi = np.full(N, -1, np.int64)
    for c in range(n_cores):
        rowsb = layouts[c]
        valid = rowsb >= 0
        pi[rowsb[valid]] = c * npc + np.where(valid)[0]
    assert (pi[np.unique(dst)] >= 0).all()

    # in-edge lists per node
    eorder = np.argsort(dst, kind="stable")
    esrc = src[eorder]; edst = dst[eorder]
    estart = np.searchsorted(edst, np.arange(N))
    eend = np.searchsorted(edst, np.arange(N), side="right")

    # slot grids: per block columns [A: TAs(b) | B: TBs(b)]
    col_off = {}
    off = 0
    for b in range(nblk):
        col_off[(b, 0)] = (off, int(TAs[b])); off += int(TAs[b])
        col_off[(b, 1)] = (off, int(TBs[b])); off += int(TBs[b])
    assert off == tot_T

    idx_mega = np.zeros((n_cores, 128, 8 * tot_T), np.int16)
    for c in range(n_cores):
        lay = layouts[c]
        for b in range(nblk):
            rowsb = lay[b * blk:(b + 1) * blk]
            for half in (0, 1):
                o, T = col_off[(b, half)]
                if T == 0:
                    continue
                base = half * half_rows
                iv = np.full((128, T), half_rows - 1, np.int64)  # pad row
                for p in range(blk):
                    v = rowsb[p]
                    if v < 0:
                        continue
                    es = esrc[estart[v]:eend[v]]
                    r = pi[es]
                    mysrc = r[r >= half_rows] if half else r[r < half_rows]
                    k = len(mysrc)
                    assert k <= T
                    iv[p, :k] = mysrc - base
                assert (iv >= 0).all() and (iv < 32768).all()
                pos = np.arange(128 * T)
                wrapped = np.zeros((16, 8 * T), np.int16)
                wrapped[pos % 16, pos // 16] = iv[pos % 128, pos // 128].astype(np.int16)
                idx_mega[c, :, o * 8:(o + T) * 8] = np.tile(wrapped, (8, 1))

    return dict(
        n_cores=n_cores, blk=blk, nblk=nblk, npc=npc, n_real=npc_real,
        half_rows=half_rows, NROWS=n_cores * npc, tot_T=tot_T, col_off=col_off,
        TA=TAs, TB=TBs, layouts=layouts, pi=pi, chunk_blocks=chunk_blocks,
        idx_mega=idx_mega,
    )


# ------------------------------------------------------------ device build

def build_program(st, H=4, DH=32, D_IN=128, D_OUT=64, nq=4):
    n_cores, blk, nblk, npc = st["n_cores"], st["blk"], st["nblk"], st["npc"]
    tot_T, col_off = st["tot_T"], st["col_off"]
    HD = H * DH                      # 128
    REC = 256                        # fp16 elems per L0/L1 record (512B)
    REC2 = 128                       # fp16 elems per L2 record (256B)
    NROWS = st["NROWS"]
    half_A = st["half_rows"]
    chunk_blocks = st["chunk_blocks"]

    nc = bacc.Bacc(None, target_bir_lowering=False, num_swdge_queues=nq)

    feats_own = nc.dram_tensor("feats_own", [npc, D_IN], F32, kind="ExternalInput")
    idx_in = nc.dram_tensor("idx_mega", [128, 8 * tot_T], I16, kind="ExternalInput")
    W0_in = nc.dram_tensor("W0", [HD, D_IN], F32, kind="ExternalInput")
    W1_in = nc.dram_tensor("W1", [HD, HD], F32, kind="ExternalInput")
    W2_in = nc.dram_tensor("W2", [D_OUT, HD], F32, kind="ExternalInput")
    M0_in = nc.dram_tensor("M0", [HD, HD + 8], F32, kind="ExternalInput")
    M1_in = nc.dram_tensor("M1", [HD, HD + 8], F32, kind="ExternalInput")
    M2_in = nc.dram_tensor("M2", [D_OUT, D_OUT + 2], F32, kind="ExternalInput")
    b0_in = nc.dram_tensor("b0", [1, HD], F32, kind="ExternalInput")
    b1_in = nc.dram_tensor("b1", [1, HD], F32, kind="ExternalInput")
    b2_in = nc.dram_tensor("b2", [1, D_OUT], F32, kind="ExternalInput")
    ones_in = nc.dram_tensor("ones", [1, 128], F32, kind="ExternalInput")
    out_t = nc.dram_tensor("out", [npc, D_OUT], F32, kind="ExternalOutput")

    LCFG = [  # per layer: rec elems (fp16), feat width, heads, el col, wcat
        dict(rec=REC, fw=HD, h=H, elc=HD, wcat=HD + 8),
        dict(rec=REC, fw=HD, h=H, elc=HD, wcat=HD + 8),
        dict(rec=REC2, fw=D_OUT, h=1, elc=D_OUT, wcat=D_OUT + 2),
    ]

    with tile.TileContext(nc) as tc:
        with tc.tile_pool(name="const", bufs=1) as cpool, \
             tc.tile_pool(name="dram", bufs=1, space="DRAM") as dram, \
             tc.tile_pool(name="xblk", bufs=1) as xpool, \
             tc.tile_pool(name="gat", bufs=6) as gpool, \
             tc.tile_pool(name="small", bufs=4) as spool, \
             tc.tile_pool(name="psum", bufs=2, space="PSUM") as ppool, \
             tc.tile_pool(name="psum1", bufs=2, space="PSUM") as ppool2:

            # ---------------- constants / prologue
            idx_sb = cpool.tile([128, 8 * tot_T], I16)
            nc.sync.dma_start(out=idx_sb[:], in_=idx_in[:])
            ident = cpool.tile([128, 128], F32)
            make_identity(nc, ident[:])
            pad_el = cpool.tile([blk, 8], F16)
            nc.vector.memset(pad_el[:], float(PAD_EL))

            wins = [W0_in, W1_in, W2_in]
            mins = [M0_in, M1_in, M2_in]
            bins = [b0_in, b1_in, b2_in]
            wcat_sb, brep_sb = [], []
            with tc.tile_pool(name="prolog", bufs=1) as plpool:
                ones_sb = plpool.tile([1, 128], F32)
                nc.sync.dma_start(out=ones_sb[:], in_=ones_in[:])
                for L in range(3):
                    cfg = LCFG[L]
                    kdim = wins[L].shape[0]
                    w_sb = plpool.tile([kdim, wins[L].shape[1]], F32, tag=f"wld{L}", name=f"wld{L}")
                    nc.sync.dma_start(out=w_sb[:], in_=wins[L][:])
                    m_sb = plpool.tile([kdim, cfg["wcat"]], F32, tag=f"mld{L}", name=f"mld{L}")
                    nc.sync.dma_start(out=m_sb[:], in_=mins[L][:])
                    wc_ps = ppool.tile([wins[L].shape[1], cfg["wcat"]], F32, space="PSUM", tag="wcps")
                    nc.tensor.matmul(out=wc_ps[:], lhsT=w_sb[:], rhs=m_sb[:], start=True, stop=True)
                    wc = cpool.tile([wins[L].shape[1], cfg["wcat"]], F32, tag=f"wcat{L}", name=f"wcat{L}")
                    nc.vector.tensor_copy(out=wc[:], in_=wc_ps[:])
                    wcat_sb.append(wc)

                    b_sb = plpool.tile([1, cfg["fw"]], F32, tag=f"bld{L}", name=f"bld{L}")
                    nc.sync.dma_start(out=b_sb[:], in_=bins[L][:])
                    br_ps = ppool.tile([128, cfg["fw"]], F32, space="PSUM", tag="brps")
                    nc.tensor.matmul(out=br_ps[:], lhsT=ones_sb[:], rhs=b_sb[:], start=True, stop=True)
                    br = cpool.tile([128, cfg["fw"]], F32, tag=f"brep{L}", name=f"brep{L}")
                    nc.vector.tensor_copy(out=br[:], in_=br_ps[:])
                    brep_sb.append(br)

            # persistent per-block x tiles and er tiles
            x_tiles = [xpool.tile([128, HD], F32, tag=f"x{b}", name=f"xblk{b}") for b in range(nblk)]
            er_tiles = [xpool.tile([128, H], F16, tag=f"er{b}", name=f"erblk{b}") for b in range(nblk)]
            for b in range(nblk):
                if st["TA"][b] + st["TB"][b] == 0:
                    nc.vector.memset(x_tiles[b][:], 0)

            # record staging tiles (pad cols zeroed once)
            rec_tiles = [xpool.tile([128, REC], F16, tag=f"rec{i}", name=f"rect{i}") for i in range(2)]
            for i in range(2):
                nc.vector.memset(rec_tiles[i][:], 0)

            slabs = [dram.tile([npc, REC], F16, tag="slab0", name="slab0"),
                     dram.tile([npc, REC], F16, tag="slab1", name="slab1"),
                     dram.tile([npc, REC2], F16, tag="slab2", name="slab2")]
            tables = [dram.tile([NROWS, REC], F16, tag="tab0", name="tab0", addr_space="Shared"),
                      dram.tile([NROWS, REC], F16, tag="tab1", name="tab1", addr_space="Shared"),
                      dram.tile([NROWS, REC2], F16, tag="tab2", name="tab2", addr_space="Shared")]

            n_real = st["n_real"]

            def dense_block(L, b):
                """Record rows for owned block b of layer L (PE + copies)."""
                cfg = LCFG[L]
                if L == 0:
                    x_sb = spool.tile([128, D_IN], F32, tag="xts", name="x0ld")
                    nc.sync.dma_start(out=x_sb[:], in_=feats_own[b * blk:(b + 1) * blk, :])
                else:
                    x_sb = x_tiles[b]
                xt_ps = ppool.tile([128, 128], F32, space="PSUM", tag="xtps", name="xt_ps")
                nc.tensor.transpose(out=xt_ps[:], in_=x_sb[:], identity=ident[:])
                xt_sb = spool.tile([128, 128], F32, tag="xts", name="xt_sb")
                if b % 2 == 0:
                    nc.scalar.copy(out=xt_sb[:], in_=xt_ps[:])
                else:
                    nc.vector.tensor_copy(out=xt_sb[:], in_=xt_ps[:])
                y_ps = ppool2.tile([128, cfg["wcat"]], F32, space="PSUM", tag="yps", name="y_ps")
                nc.tensor.matmul(out=y_ps[:], lhsT=xt_sb[:], rhs=wcat_sb[L][:], start=True, stop=True)
                recb = rec_tiles[b % 2]
                # feat + el (er col dropped for L2; harmless extra er cols for L0/L1)
                ncopy = cfg["elc"] + cfg["h"]
                nc.scalar.copy(out=recb[:, 0:ncopy], in_=y_ps[:, 0:ncopy])
                nc.vector.tensor_copy(out=er_tiles[b][:, 0:cfg["h"]],
                                      in_=y_ps[:, cfg["elc"] + cfg["h"]:cfg["elc"] + 2 * cfg["h"]])
                nc.sync.dma_start(out=slabs[L][b * blk:(b + 1) * blk, :],
                                  in_=recb[:, 0:cfg["rec"]])

            def finish_chunk(L, k):
                """Pad-row el poison (last chunk) + AllGather chunk k."""
                cfg = LCFG[L]
                b0, b1 = chunk_blocks[k]
                if k == NCHUNK - 1:
                    nc.sync.dma_start(
                        out=slabs[L][n_real:npc, cfg["elc"]:cfg["elc"] + cfg["h"]],
                        in_=pad_el[0:npc - n_real, 0:cfg["h"]])
                if (b0, b1) == (0, nblk):
                    out_ap = tables[L][:].opt()
                else:
                    out_v = tables[L][:].rearrange("(c r) e -> c r e", c=n_cores)
                    out_ap = out_v[:, b0 * blk:b1 * blk, :]
                nc.gpsimd.collective_compute(
                    "AllGather", mybir.AluOpType.bypass,
                    replica_groups=[list(range(n_cores))],
                    ins=[slabs[L][b0 * blk:b1 * blk, :].opt()],
                    outs=[out_ap],
                )

            def chunk_of_block(b):
                for k, (b0, b1) in enumerate(chunk_blocks):
                    if b0 <= b < b1:
                        return k, b == b1 - 1
                raise AssertionError

            for b in range(nblk):
                dense_block(0, b)
                k, last = chunk_of_block(b)
                if last:
                    finish_chunk(0, k)

            for L in range(3):
                cfg = LCFG[L]
                h, fw, rec = cfg["h"], cfg["fw"], cfg["rec"]
                elc = cfg["elc"]
                dh = fw // h
                tabA = tables[L][0:half_A, :]
                tabB = tables[L][half_A:NROWS, :]

                gcount = 0
                for b in range(nblk):
                    oA, TA_ = col_off[(b, 0)]
                    oB, TB_ = col_off[(b, 1)]
                    T = TA_ + TB_
                    if T == 0:
                        continue
                    grid = gpool.tile([128, T, rec], F16, tag="grid")
                    for (hf, o, Th) in ((0, oA, TA_), (1, oB, TB_)):
                        goff = 0 if hf == 0 else TA_
                        for t0 in range(0, Th, 16):
                            tw = min(16, Th - t0)
                            nc.gpsimd.dma_gather(
                                out_ap=grid[:, goff + t0:goff + t0 + tw, :],
                                in_ap=(tabA if hf == 0 else tabB),
                                idxs_ap=idx_sb[:, (o + t0) * 8:(o + t0 + tw) * 8],
                                num_idxs=128 * tw,
                                num_idxs_reg=128 * tw,
                                elem_size=rec,
                                single_packet=(tw <= 8),
                                queue_num=gcount % nq,
                            )
                            gcount += 1

                    # e = el + er (bcast over slots), in place on el cols
                    el = grid[:, :, elc:elc + h]                  # [128,T,h] f16
                    erb = er_tiles[b][:, 0:h].unsqueeze(1).to_broadcast([128, T, h])
                    nc.vector.tensor_tensor(out=el, in0=el, in1=erb, op=mybir.AluOpType.add)
                    # leaky relu
                    nc.vector.scalar_tensor_tensor(
                        out=el, in0=el, scalar=0.2, in1=el,
                        op0=mybir.AluOpType.mult, op1=mybir.AluOpType.max)
                    # rowmax over slots, subtract (exact edge softmax)
                    mx = spool.tile([128, h], F16, tag="mx")
                    elT = el.rearrange("p t h -> p h t")
                    nc.vector.tensor_reduce(out=mx[:], in_=elT, axis=mybir.AxisListType.X,
                                            op=mybir.AluOpType.max)
                    mxb = mx[:, 0:h].unsqueeze(1).to_broadcast([128, T, h])
                    nc.vector.tensor_tensor(out=el, in0=el, in1=mxb, op=mybir.AluOpType.subtract)
                    # ex = exp(e - max)
                    nc.scalar.activation(out=el, in_=el, func=mybir.ActivationFunctionType.Exp)
                    # denominator + reciprocal
                    den = spool.tile([128, h], F32, tag="den")
                    nc.vector.tensor_reduce(out=den[:], in_=elT, axis=mybir.AxisListType.X,
                                            op=mybir.AluOpType.add)
                    rden = spool.tile([128, h], F32, tag="rden")
                    nc.vector.reciprocal(out=rden[:], in_=den[:])
                    # unnormalized msg = feat * ex, in place on the grid so
                    # every AP walks contiguous 256B runs (strided views
                    # halve DVE throughput)
                    feat3 = grid[:, :, 0:fw].rearrange("p t (h d) -> p t h d", h=h)
                    exb = el.unsqueeze(3).to_broadcast([128, T, h, dh])
                    nc.vector.tensor_tensor(out=feat3, in0=feat3, in1=exb, op=mybir.AluOpType.mult)
                    # slot reduction as a binary tree of contiguous adds
                    Tr = T
                    while Tr > 1:
                        hlf = Tr // 2
                        dsts = grid[:, 0:hlf, 0:fw]
                        srcs = grid[:, Tr - hlf:Tr, 0:fw]
                        nc.vector.tensor_tensor(out=dsts, in0=dsts, in1=srcs, op=mybir.AluOpType.add)
                        Tr -= hlf
                    agg = spool.tile([128, fw], F32, tag="agg")
                    nc.vector.tensor_copy(out=agg[:], in_=grid[:, 0, 0:fw])
                    # x = (relu of) agg * rden + bias
                    rdb = rden[:, 0:h].unsqueeze(2).to_broadcast([128, h, dh])
                    agg_v = agg[:].rearrange("p (h d) -> p h d", h=h)
                    nc.vector.tensor_tensor(out=agg_v, in0=agg_v, in1=rdb, op=mybir.AluOpType.mult)
                    if L < 2:
                        nc.vector.tensor_tensor(out=agg[:], in0=agg[:], in1=brep_sb[L][:],
                                                op=mybir.AluOpType.add)
                        nc.scalar.activation(out=x_tiles[b][:], in_=agg[:],
                                             func=mybir.ActivationFunctionType.Relu)
                        dense_block(L + 1, b)
                        k, lastb = chunk_of_block(b)
                        if lastb:
                            finish_chunk(L + 1, k)
                    else:
                        ob = spool.tile([128, D_OUT], F32, tag="ob")
                        nc.vector.tensor_tensor(out=ob[:], in0=agg[:], in1=brep_sb[2][:],
                                                op=mybir.AluOpType.add)
                        nc.sync.dma_start(out=out_t[b * blk:(b + 1) * blk, :], in_=ob[:])

                if L < 2:
                    for b in range(nblk):
                        if st["TA"][b] + st["TB"][b] == 0:
                            dense_block(L + 1, b)
                            k, lastb = chunk_of_block(b)
                            if lastb:
                                finish_chunk(L + 1, k)

    nc.compile()
    return nc


# ------------------------------------------------------------------- runner

def make_in_maps(st, inputs, H=4, DH=32, D_IN=128, D_OUT=64):
    n_cores, npc = st["n_cores"], st["npc"]
    feats = np.asarray(inputs["feats"], np.float32)
    HD = H * DH

    def mk_M(al, ar, d):
        h = al.shape[0]
        M = np.zeros((d, d + 2 * h), np.float32)
        M[:, :d] = np.eye(d, dtype=np.float32)
        for hh in range(h):
            M[hh * (d // h):(hh + 1) * (d // h), d + hh] = al[hh]
            M[hh * (d // h):(hh + 1) * (d // h), d + h + hh] = ar[hh]
        return M

    M0 = mk_M(np.asarray(inputs["al0"]), np.asarray(inputs["ar0"]), HD)
    M1 = mk_M(np.asarray(inputs["al1"]), np.asarray(inputs["ar1"]), HD)
    M2 = mk_M(np.asarray(inputs["al2"]), np.asarray(inputs["ar2"]), D_OUT)

    shared = dict(
        W0=np.asarray(inputs["W0"], np.float32),
        W1=np.asarray(inputs["W1"], np.float32),
        W2=np.asarray(inputs["W2"], np.float32),
        M0=M0, M1=M1, M2=M2,
        b0=np.asarray(inputs["b0"], np.float32).reshape(1, -1),
        b1=np.asarray(inputs["b1"], np.float32).reshape(1, -1),
        b2=np.asarray(inputs["b2"], np.float32).reshape(1, -1),
        ones=np.ones((1, 128), np.float32),
    )
    in_maps = []
    for c in range(n_cores):
        lay = st["layouts"][c]
        fo = np.zeros((npc, D_IN), np.float32)
        valid = lay >= 0
        fo[valid] = feats[lay[valid]]
        m = dict(shared)
        m["feats_own"] = fo
        m["idx_mega"] = st["idx_mega"][c]
        in_maps.append(m)
    return in_maps


def assemble_output(st, results, N, D_OUT=64):
    out = np.zeros((N, D_OUT), np.float32)
    for c, r in enumerate(results):
        lay = st["layouts"][c]
        valid = lay >= 0
        out[lay[valid]] = r["out"][valid]
    return out


# =================================================================== kernel

_CACHE = {}
LAST_EXEC_NS = None
LAST_TRACE = None
LAST_RES = None


def kernel(**inputs):
    """Full-input GAT forward on 8 NeuronCores; returns [50000, 64] f32."""
    global LAST_EXEC_NS, LAST_TRACE, LAST_RES
    from concourse.bass_utils import run_bass_kernel_spmd

    N = 50000
    src = np.asarray(inputs["src"])
    dst = np.asarray(inputs["dst"])

    if "prog" not in _CACHE:
        st = prep_host(src, dst, N, n_cores=8)
        nc = build_program(st)
        _CACHE["prog"] = (st, nc)
    st, nc = _CACHE["prog"]

    in_maps = make_in_maps(st, inputs)
    trace = os.environ.get("GAT_TRACE", "0") == "1"
    res = run_bass_kernel_spmd(nc, in_maps, core_ids=list(range(8)), trace=trace)
    LAST_EXEC_NS = res.exec_time_ns
    LAST_RES = res
    if res.instructions_and_trace:
        LAST_TRACE = res.instructions_and_trace[1]
    return assemble_output(st, [res.results[c] for c in range(8)], N)


# revision 23
# speedup vs baseline: 1.4261x; 1.0924x over previous
"""GAT (3-layer, 4/4/1 heads) on 8 TRN2 NeuronCores.

Design (per core, SPMD one program):
- Nodes dealt to cores by degree (snake); within a core, nodes 2D-packed by
  (deg_A, deg_B) into 128-node blocks, blocks sorted by slot width desc so
  the shared (max-over-cores) slot profile stays tight.
- Per-edge records are fp16, exactly 256B: since el = al.feat is linear in
  feat, the record stores a permuted basis of feat where coordinate 31 of
  each head block holds el; the displaced feat coordinate is reconstructed
  after aggregation with one small weighted reduce (host bakes the
  permutation + al into the layer's M matrix and into the next layer's W).
- Records gathered per edge via SWDGE dma_gather (int16 idx; table split in
  two halves = cores 0-3 / 4-7 rows), 16 slots per instruction, queues
  round-robin, 10 grid buffers in flight (gather is descriptor-rate-bound
  and latency-sensitive).
- AllGather per layer into a Shared DRAM table (~260GB/s).
- Edge phase per block: copy el slots to a compact tile; e = el+er ->
  leaky -> rowmax-subtract -> exp -> denom -> alpha = ex/den; then
  grid *= alpha in place (contiguous APs) and a binary tree of contiguous
  adds reduces slots; bias+relu -> x (fp16); dense phase of layer L+1
  (PE matmul) is interleaved per block.
"""

import os
import numpy as np

import concourse.bacc as bacc
import concourse.bass as bass
import concourse.tile as tile
from concourse import mybir
from concourse.masks import make_identity

F32 = mybir.dt.float32
F16 = mybir.dt.float16
I16 = mybir.dt.int16

PAD_EL = -30000.0
NCHUNK = 1


# ---------------------------------------------------------------- host prep

def prep_host(src, dst, N, n_cores=8, blk=128):
    """Graph -> per-core slot grids (owner-half A/B split)."""
    src = np.asarray(src); dst = np.asarray(dst)
    deg = np.bincount(dst, minlength=N)

    order = np.argsort(-deg, kind="stable")
    owner = np.empty(N, np.int32)
    for i, v in enumerate(order):
        r = i % (2 * n_cores)
        owner[v] = r if r < n_cores else 2 * n_cores - 1 - r

    half_of_core = (np.arange(n_cores) >= n_cores // 2).astype(np.int32)
    src_half = half_of_core[owner[src]]
    degA = np.bincount(dst[src_half == 0], minlength=N)
    degB = np.bincount(dst[src_half == 1], minlength=N)

    core_nodes = [np.where(owner == c)[0] for c in range(n_cores)]
    npc_real = max(len(cn) for cn in core_nodes)
    assert all(len(cn) == npc_real for cn in core_nodes)
    nblk = (npc_real + blk) // blk          # >=1 pad row per core
    npc = nblk * blk

    cb = [0]
    per = (nblk + NCHUNK - 1) // NCHUNK
    for k in range(NCHUNK):
        cb.append(min(nblk, cb[-1] + per))
    chunk_blocks = [(cb[k], cb[k + 1]) for k in range(NCHUNK)]

    layouts = []
    TA = np.zeros((n_cores, nblk), np.int32)
    TB = np.zeros((n_cores, nblk), np.int32)
    for c in range(n_cores):
        cn = core_nodes[c]
        key = np.maximum(degA[cn], degB[cn]) * 1000 + np.minimum(degA[cn], degB[cn])
        cn = cn[np.argsort(-key, kind="stable")]
        lay = np.full(npc, -1, np.int64)
        lay[:len(cn)] = cn
        for b in range(nblk):
            rowsb = lay[b * blk:(b + 1) * blk]
            real = rowsb[rowsb >= 0]
            TA[c, b] = degA[real].max() if len(real) else 0
            TB[c, b] = degB[real].max() if len(real) else 0
        bo = np.argsort(-(TA[c] + TB[c]), kind="stable")
        lay = np.concatenate([lay[b * blk:(b + 1) * blk] for b in bo])
        TA[c], TB[c] = TA[c][bo], TB[c][bo]
        assert (lay[npc_real:] < 0).all(), "pad rows must stay in the last block"
        layouts.append(lay)

    TAs = TA.max(axis=0)
    TBs = TB.max(axis=0)
    tot_T = int(TAs.sum() + TBs.sum())

    half_rows = (n_cores // 2) * npc
    assert half_rows < 32768
    pi = np.full(N, -1, np.int64)
    for c in range(n_cores):
        rowsb = layouts[c]
        valid = rowsb >= 0
        pi[rowsb[valid]] = c * npc + np.where(valid)[0]
    assert (pi[np.unique(dst)] >= 0).all()

    eorder = np.argsort(dst, kind="stable")
    esrc = src[eorder]; edst = dst[eorder]
    estart = np.searchsorted(edst, np.arange(N))
    eend = np.searchsorted(edst, np.arange(N), side="right")

    col_off = {}
    off = 0
    for b in range(nblk):
        col_off[(b, 0)] = (off, int(TAs[b])); off += int(TAs[b])
        col_off[(b, 1)] = (off, int(TBs[b])); off += int(TBs[b])
    assert off == tot_T

    idx_mega = np.zeros((n_cores, 128, 8 * tot_T), np.int16)
    for c in range(n_cores):
        lay = layouts[c]
        for b in range(nblk):
            rowsb = lay[b * blk:(b + 1) * blk]
            for half in (0, 1):
                o, T = col_off[(b, half)]
                if T == 0:
                    continue
                base = half * half_rows
                iv = np.full((128, T), half_rows - 1, np.int64)  # pad row
                for p in range(blk):
                    v = rowsb[p]
                    if v < 0:
                        continue
                    r = pi[esrc[estart[v]:eend[v]]]
                    mysrc = r[r >= half_rows] if half else r[r < half_rows]
                    k = len(mysrc)
                    assert k <= T
                    iv[p, :k] = mysrc - base
                assert (iv >= 0).all() and (iv < 32768).all()
                pos = np.arange(128 * T)
                wrapped = np.zeros((16, 8 * T), np.int16)
                wrapped[pos % 16, pos // 16] = iv[pos % 128, pos // 128].astype(np.int16)
                idx_mega[c, :, o * 8:(o + T) * 8] = np.tile(wrapped, (8, 1))

    return dict(
        n_cores=n_cores, blk=blk, nblk=nblk, npc=npc, n_real=npc_real,
        half_rows=half_rows, NROWS=n_cores * npc, tot_T=tot_T, col_off=col_off,
        TA=TAs, TB=TBs, layouts=layouts, pi=pi, chunk_blocks=chunk_blocks,
        idx_mega=idx_mega,
    )


# ------------------------------------------------------------ device build

def build_program(st, H=4, DH=32, D_IN=128, D_OUT=64, nq=4):
    n_cores, blk, nblk, npc = st["n_cores"], st["blk"], st["nblk"], st["npc"]
    tot_T, col_off = st["tot_T"], st["col_off"]
    HD = H * DH                      # 128
    REC = 128                        # fp16 elems per record (256B), all layers
    NROWS = st["NROWS"]
    half_A = st["half_rows"]
    chunk_blocks = st["chunk_blocks"]

    nc = bacc.Bacc(None, target_bir_lowering=False, num_swdge_queues=nq)

    feats_own = nc.dram_tensor("feats_own", [npc, D_IN], F16, kind="ExternalInput")
    idx_in = nc.dram_tensor("idx_mega", [128, 8 * tot_T], I16, kind="ExternalInput")
    W0_in = nc.dram_tensor("W0", [HD, D_IN], F32, kind="ExternalInput")
    W1_in = nc.dram_tensor("W1", [HD, HD], F32, kind="ExternalInput")
    W2_in = nc.dram_tensor("W2", [D_OUT, HD], F32, kind="ExternalInput")
    M0_in = nc.dram_tensor("M0", [HD, HD + 4], F32, kind="ExternalInput")
    M1_in = nc.dram_tensor("M1", [HD, HD + 4], F32, kind="ExternalInput")
    M2_in = nc.dram_tensor("M2", [D_OUT, D_OUT + 2], F32, kind="ExternalInput")
    RV0_in = nc.dram_tensor("RV0", [1, HD], F32, kind="ExternalInput")
    RV1_in = nc.dram_tensor("RV1", [1, HD], F32, kind="ExternalInput")
    b0_in = nc.dram_tensor("b0", [1, HD], F32, kind="ExternalInput")
    b1_in = nc.dram_tensor("b1", [1, HD], F32, kind="ExternalInput")
    b2_in = nc.dram_tensor("b2", [1, D_OUT], F32, kind="ExternalInput")
    ones_in = nc.dram_tensor("ones", [1, 128], F32, kind="ExternalInput")
    out_t = nc.dram_tensor("out", [npc, D_OUT], F32, kind="ExternalOutput")

    # per layer: feat width, heads, el col (L2 only), wcat cols
    LCFG = [
        dict(fw=HD, h=H, wcat=HD + 4),
        dict(fw=HD, h=H, wcat=HD + 4),
        dict(fw=D_OUT, h=1, elc=D_OUT, wcat=D_OUT + 2),
    ]

    with tile.TileContext(nc) as tc:
        with tc.tile_pool(name="const", bufs=1) as cpool, \
             tc.tile_pool(name="dram", bufs=1, space="DRAM") as dram, \
             tc.tile_pool(name="xblk", bufs=1) as xpool, \
             tc.tile_pool(name="gat", bufs=10) as gpool, \
             tc.tile_pool(name="et", bufs=4) as epool, \
             tc.tile_pool(name="small", bufs=4) as spool, \
             tc.tile_pool(name="psum", bufs=2, space="PSUM") as ppool, \
             tc.tile_pool(name="psum1", bufs=2, space="PSUM") as ppool2:

            # ---------------- constants / prologue
            idx_sb = cpool.tile([128, 8 * tot_T], I16)
            nc.sync.dma_start(out=idx_sb[:], in_=idx_in[:])
            ident = cpool.tile([128, 128], F16)
            make_identity(nc, ident[:])
            pad_el = cpool.tile([blk, 8], F16)
            nc.vector.memset(pad_el[:], PAD_EL)

            wins = [W0_in, W1_in, W2_in]
            mins = [M0_in, M1_in, M2_in]
            bins = [b0_in, b1_in, b2_in]
            wcat_sb, brep_sb, wrep_sb = [], [], []
            with tc.tile_pool(name="prolog", bufs=1) as plpool:
                ones_sb = plpool.tile([1, 128], F32)
                nc.sync.dma_start(out=ones_sb[:], in_=ones_in[:])
                for L in range(3):
                    cfg = LCFG[L]
                    kdim = wins[L].shape[0]
                    w_sb = plpool.tile([kdim, wins[L].shape[1]], F32, tag=f"wld{L}", name=f"wld{L}")
                    nc.sync.dma_start(out=w_sb[:], in_=wins[L][:])
                    m_sb = plpool.tile([kdim, cfg["wcat"]], F32, tag=f"mld{L}", name=f"mld{L}")
                    nc.sync.dma_start(out=m_sb[:], in_=mins[L][:])
                    wc_ps = ppool.tile([wins[L].shape[1], cfg["wcat"]], F32, space="PSUM", tag="wcps")
                    nc.tensor.matmul(out=wc_ps[:], lhsT=w_sb[:], rhs=m_sb[:], start=True, stop=True)
                    wc = cpool.tile([wins[L].shape[1], cfg["wcat"]], F16, tag=f"wcat{L}", name=f"wcat{L}")
                    nc.vector.tensor_copy(out=wc[:], in_=wc_ps[:])
                    wcat_sb.append(wc)

                    b_sb = plpool.tile([1, cfg["fw"]], F32, tag=f"bld{L}", name=f"bld{L}")
                    nc.sync.dma_start(out=b_sb[:], in_=bins[L][:])
                    br_ps = ppool.tile([128, cfg["fw"]], F32, space="PSUM", tag="brps")
                    nc.tensor.matmul(out=br_ps[:], lhsT=ones_sb[:], rhs=b_sb[:], start=True, stop=True)
                    br = cpool.tile([128, cfg["fw"]], F16 if L < 2 else F32,
                                    tag=f"brep{L}", name=f"brep{L}")
                    nc.vector.tensor_copy(out=br[:], in_=br_ps[:])
                    brep_sb.append(br)

                    if L < 2:
                        rv_sb = plpool.tile([1, HD], F32, tag=f"rvld{L}", name=f"rvld{L}")
                        nc.sync.dma_start(out=rv_sb[:], in_=(RV0_in if L == 0 else RV1_in)[:])
                        wr_ps = ppool.tile([128, HD], F32, space="PSUM", tag="brps")
                        nc.tensor.matmul(out=wr_ps[:], lhsT=ones_sb[:], rhs=rv_sb[:], start=True, stop=True)
                        wr = cpool.tile([128, HD], F16, tag=f"wrep{L}", name=f"wrep{L}")
                        nc.vector.tensor_copy(out=wr[:], in_=wr_ps[:])
                        wrep_sb.append(wr)

            x_tiles = [xpool.tile([128, HD], F16, tag=f"x{b}", name=f"xblk{b}") for b in range(nblk)]
            er_tiles = [xpool.tile([128, H], F16, tag=f"er{b}", name=f"erblk{b}") for b in range(nblk)]
            rec_tiles = [xpool.tile([128, REC], F16, tag=f"rec{i}", name=f"rect{i}") for i in range(2)]

            slabs = [dram.tile([npc, REC], F16, tag=f"slab{i}", name=f"slab{i}") for i in range(3)]
            tables = [dram.tile([NROWS, REC], F16, tag=f"tab{i}", name=f"tab{i}",
                                addr_space="Shared") for i in range(3)]

            n_real = st["n_real"]

            def el_view(L, ap, T):
                """[128, T, h] view of the el slots inside a record range."""
                cfg = LCFG[L]
                h, fw = cfg["h"], cfg["fw"]
                if L < 2:
                    return ap.rearrange("p t (h d) -> p t h d", h=h)[:, :, :, DH - 1:DH].squeeze(3)
                return ap[:, :, cfg["elc"]:cfg["elc"] + 1].rearrange("p t e -> p t e")

            def dense_block(L, b):
                cfg = LCFG[L]
                if L == 0:
                    x_sb = spool.tile([128, D_IN], F16, tag="xts", name="x0ld")
                    nc.sync.dma_start(out=x_sb[:], in_=feats_own[b * blk:(b + 1) * blk, :])
                else:
                    x_sb = x_tiles[b]
                xt_ps = ppool.tile([128, 128], F16, space="PSUM", tag="xtps", name="xt_ps")
                nc.tensor.transpose(out=xt_ps[:], in_=x_sb[:], identity=ident[:])
                xt_sb = spool.tile([128, 128], F16, tag="xts", name="xt_sb")
                if b % 2 == 0:
                    nc.scalar.copy(out=xt_sb[:], in_=xt_ps[:])
                else:
                    nc.vector.tensor_copy(out=xt_sb[:], in_=xt_ps[:])
                y_ps = ppool2.tile([128, cfg["wcat"]], F32, space="PSUM", tag="yps", name="y_ps")
                nc.tensor.matmul(out=y_ps[:], lhsT=xt_sb[:], rhs=wcat_sb[L][:], start=True, stop=True)
                recb = rec_tiles[b % 2]
                ncopy = cfg["fw"] + (1 if L == 2 else 0)   # L2: feat + el col
                nc.scalar.copy(out=recb[:, 0:ncopy], in_=y_ps[:, 0:ncopy])
                erc = cfg["fw"] + (1 if L == 2 else 0)
                nc.vector.tensor_copy(out=er_tiles[b][:, 0:cfg["h"]],
                                      in_=y_ps[:, erc:erc + cfg["h"]])
                nc.sync.dma_start(out=slabs[L][b * blk:(b + 1) * blk, :],
                                  in_=recb[:, 0:REC])

            def finish_chunk(L, k):
                cfg = LCFG[L]
                b0, b1 = chunk_blocks[k]
                if k == NCHUNK - 1:
                    # poison pad rows' el slots
                    npad = npc - n_real
                    if L < 2:
                        pv = slabs[L][n_real:npc, :].rearrange(
                            "r (h d) -> r h d", h=cfg["h"])[:, :, DH - 1:DH]
                        nc.sync.dma_start(out=pv, in_=pad_el[0:npad, 0:cfg["h"]].unsqueeze(2))
                    else:
                        nc.sync.dma_start(out=slabs[L][n_real:npc, cfg["elc"]:cfg["elc"] + 1],
                                          in_=pad_el[0:npad, 0:1])
                if (b0, b1) == (0, nblk):
                    out_ap = tables[L][:].opt()
                else:
                    out_v = tables[L][:].rearrange("(c r) e -> c r e", c=n_cores)
                    out_ap = out_v[:, b0 * blk:b1 * blk, :]
                nc.gpsimd.collective_compute(
                    "AllGather", mybir.AluOpType.bypass,
                    replica_groups=[list(range(n_cores))],
                    ins=[slabs[L][b0 * blk:b1 * blk, :].opt()],
                    outs=[out_ap],
                )

            def chunk_of_block(b):
                for k, (b0, b1) in enumerate(chunk_blocks):
                    if b0 <= b < b1:
                        return k, b == b1 - 1
                raise AssertionError

            for b in range(nblk):
                dense_block(0, b)
                k, last = chunk_of_block(b)
                if last:
                    finish_chunk(0, k)

            for L in range(3):
                cfg = LCFG[L]
                h, fw = cfg["h"], cfg["fw"]
                dh = fw // h
                tabA = tables[L][0:half_A, :]
                tabB = tables[L][half_A:NROWS, :]

                gcount = 0
                for b in range(nblk):
                    oA, TA_ = col_off[(b, 0)]
                    oB, TB_ = col_off[(b, 1)]
                    T = TA_ + TB_
                    if T == 0:
                        continue
                    grid = gpool.tile([128, T, REC], F16, tag="grid")
                    for (hf, o, Th) in ((0, oA, TA_), (1, oB, TB_)):
                        goff = 0 if hf == 0 else TA_
                        for t0 in range(0, Th, 16):
                            tw = min(16, Th - t0)
                            nc.gpsimd.dma_gather(
                                out_ap=grid[:, goff + t0:goff + t0 + tw, :],
                                in_ap=(tabA if hf == 0 else tabB),
                                idxs_ap=idx_sb[:, (o + t0) * 8:(o + t0 + tw) * 8],
                                num_idxs=128 * tw,
                                num_idxs_reg=128 * tw,
                                elem_size=REC,
                                single_packet=(tw <= 8),
                                queue_num=gcount % nq,
                            )
                            gcount += 1

                    # compact el tile; chain runs on contiguous APs
                    et = epool.tile([128, T, h], F16, tag="et")
                    nc.vector.tensor_copy(out=et[:], in_=el_view(L, grid[:, :, 0:REC], T))
                    erb = er_tiles[b][:, 0:h].unsqueeze(1).to_broadcast([128, T, h])
                    nc.vector.tensor_tensor(out=et[:], in0=et[:], in1=erb, op=mybir.AluOpType.add)
                    nc.vector.scalar_tensor_tensor(
                        out=et[:], in0=et[:], scalar=0.2, in1=et[:],
                        op0=mybir.AluOpType.mult, op1=mybir.AluOpType.max)
                    mx = spool.tile([128, h], F16, tag="mx")
                    etT = et[:].rearrange("p t h -> p h t")
                    with nc.allow_low_precision(reason="fp16 softmax stats: <=64 bounded terms"):
                        nc.vector.tensor_reduce(out=mx[:], in_=etT, axis=mybir.AxisListType.X,
                                                op=mybir.AluOpType.max)
                    mxb = mx[:, 0:h].unsqueeze(1).to_broadcast([128, T, h])
                    nc.vector.tensor_tensor(out=et[:], in0=et[:], in1=mxb,
                                            op=mybir.AluOpType.subtract)
                    nc.scalar.activation(out=et[:], in_=et[:],
                                         func=mybir.ActivationFunctionType.Exp)
                    den = spool.tile([128, h], F16, tag="den")
                    with nc.allow_low_precision(reason="fp16 softmax stats: <=64 bounded terms"):
                        nc.vector.tensor_reduce(out=den[:], in_=etT, axis=mybir.AxisListType.X,
                                                op=mybir.AluOpType.add)
                    rden = spool.tile([128, h], F16, tag="rden")
                    with nc.allow_low_precision(reason="fp16 softmax denom reciprocal, den in [1,64]"):
                        nc.vector.reciprocal(out=rden[:], in_=den[:])
                    rdb = rden[:, 0:h].unsqueeze(1).to_broadcast([128, T, h])
                    nc.vector.tensor_tensor(out=et[:], in0=et[:], in1=rdb,
                                            op=mybir.AluOpType.mult)
                    # grid *= alpha (in place, contiguous), then tree-reduce
                    feat4 = grid[:, :, 0:fw].rearrange("p t (h d) -> p t h d", h=h)
                    exb = et[:].unsqueeze(3).to_broadcast([128, T, h, dh])
                    nc.vector.tensor_tensor(out=feat4, in0=feat4, in1=exb,
                                            op=mybir.AluOpType.mult)
                    Tr = T
                    while Tr > 1:
                        hlf = Tr // 2
                        dsts = grid[:, 0:hlf, 0:fw]
                        srcs = grid[:, Tr - hlf:Tr, 0:fw]
                        nc.vector.tensor_tensor(out=dsts, in0=dsts, in1=srcs,
                                                op=mybir.AluOpType.add)
                        Tr -= hlf
                    aggv = grid[:, 0, 0:fw]
                    if L < 2:
                        # reconstruct the displaced feat coordinate per head
                        t31 = spool.tile([128, fw], F16, tag="t31")
                        nc.vector.tensor_tensor(out=t31[:], in0=aggv, in1=wrep_sb[L][:],
                                                op=mybir.AluOpType.mult)
                        x31 = spool.tile([128, h], F16, tag="x31")
                        with nc.allow_low_precision(reason="fp16 32-term reconstruction dot"):
                            nc.vector.tensor_reduce(
                                out=x31[:], in_=t31[:].rearrange("p (h d) -> p h d", h=h),
                                axis=mybir.AxisListType.X, op=mybir.AluOpType.add)
                        a3 = grid[:, 0, 0:fw].rearrange("p (h d) -> p h d", h=h)[:, :, DH - 1:DH]
                        nc.vector.tensor_copy(out=a3, in_=x31[:].unsqueeze(2))
                        nc.vector.tensor_tensor(out=aggv, in0=aggv, in1=brep_sb[L][:],
                                                op=mybir.AluOpType.add)
                        nc.scalar.activation(out=x_tiles[b][:], in_=aggv,
                                             func=mybir.ActivationFunctionType.Relu)
                        if b == nblk - 1 and npc > n_real:
                            # pad rows reconstruct to +-inf (poisoned el agg
                            # times 1/al); zero them so the next dense matmul
                            # stays finite. predicate (base - p >= 0) keeps
                            # real partitions, fills pads with 0.
                            nc.gpsimd.affine_select(
                                out=x_tiles[b][:], in_=x_tiles[b][:],
                                compare_op=mybir.AluOpType.is_ge,
                                fill=0.0,
                                base=n_real - (nblk - 1) * blk - 1,
                                pattern=[[0, HD]],
                                channel_multiplier=-1,
                            )
                        dense_block(L + 1, b)
                        k, lastb = chunk_of_block(b)
                        if lastb:
                            finish_chunk(L + 1, k)
                    else:
                        ob = spool.tile([128, D_OUT], F32, tag="ob")
                        nc.vector.tensor_copy(out=ob[:], in_=aggv)
                        nc.vector.tensor_tensor(out=ob[:], in0=ob[:], in1=brep_sb[2][:],
                                                op=mybir.AluOpType.add)
                        nc.sync.dma_start(out=out_t[b * blk:(b + 1) * blk, :], in_=ob[:])

                if L < 2:
                    for b in range(nblk):
                        if st["TA"][b] + st["TB"][b] == 0:
                            dense_block(L + 1, b)
                            k, lastb = chunk_of_block(b)
                            if lastb:
                                finish_chunk(L + 1, k)

    nc.compile()
    return nc


# ------------------------------------------------------------------- runner

def make_in_maps(st, inputs, H=4, DH=32, D_IN=128, D_OUT=64):
    n_cores, npc = st["n_cores"], st["npc"]
    feats = np.asarray(inputs["feats"], np.float32)
    HD = H * DH

    def mk_basis(al, ar):
        """M' [HD, HD+4] mapping y=Wx -> [rec | er], the reconstruction
        coefficient vector rv [HD], and the feat-coord permutation perm such
        that x_rec[c] = x_feat[perm[c]]."""
        h, dh = al.shape
        M = np.zeros((HD, HD + h), np.float32)
        rv = np.zeros(HD, np.float32)
        perm = np.arange(HD)
        for hh in range(h):
            base = hh * dh
            j = int(np.argmax(np.abs(al[hh])))
            sig = np.arange(dh)
            sig[j], sig[dh - 1] = sig[dh - 1], sig[j]
            for d_ in range(dh - 1):
                M[base + sig[d_], base + d_] = 1.0
            M[base:base + dh, base + dh - 1] = al[hh]
            M[base:base + dh, HD + hh] = ar[hh]
            perm[base:base + dh] = base + sig
            rv[base:base + dh - 1] = -al[hh][sig[:dh - 1]] / al[hh][j]
            rv[base + dh - 1] = 1.0 / al[hh][j]
        return M, rv, perm

    al0 = np.asarray(inputs["al0"]); ar0 = np.asarray(inputs["ar0"])
    al1 = np.asarray(inputs["al1"]); ar1 = np.asarray(inputs["ar1"])
    al2 = np.asarray(inputs["al2"]); ar2 = np.asarray(inputs["ar2"])

    M0, rv0, perm0 = mk_basis(al0, ar0)
    M1, rv1, perm1 = mk_basis(al1, ar1)

    M2 = np.zeros((D_OUT, D_OUT + 2), np.float32)
    M2[:, :D_OUT] = np.eye(D_OUT, dtype=np.float32)
    M2[:, D_OUT] = al2[0]
    M2[:, D_OUT + 1] = ar2[0]

    W0 = np.asarray(inputs["W0"], np.float32)
    W1 = np.asarray(inputs["W1"], np.float32)[:, perm0]
    W2 = np.asarray(inputs["W2"], np.float32)[:, perm1]
    b0 = np.asarray(inputs["b0"], np.float32)[perm0]
    b1 = np.asarray(inputs["b1"], np.float32)[perm1]

    shared = dict(
        W0=W0, W1=W1, W2=W2, M0=M0, M1=M1, M2=M2,
        RV0=rv0.reshape(1, -1), RV1=rv1.reshape(1, -1),
        b0=b0.reshape(1, -1), b1=b1.reshape(1, -1),
        b2=np.asarray(inputs["b2"], np.float32).reshape(1, -1),
        ones=np.ones((1, 128), np.float32),
    )
    in_maps = []
    for c in range(n_cores):
        lay = st["layouts"][c]
        fo = np.zeros((npc, D_IN), np.float16)
        valid = lay >= 0
        fo[valid] = feats[lay[valid]].astype(np.float16)
        m = dict(shared)
        m["feats_own"] = fo
        m["idx_mega"] = st["idx_mega"][c]
        in_maps.append(m)
    return in_maps


def assemble_output(st, results, N, D_OUT=64):
    out = np.zeros((N, D_OUT), np.float32)
    for c, r in enumerate(results):
        lay = st["layouts"][c]
        valid = lay >= 0
        out[lay[valid]] = r["out"][valid]
    return out


# =================================================================== kernel

_CACHE = {}
LAST_EXEC_NS = None
LAST_TRACE = None
LAST_RES = None


def kernel(**inputs):
    """Full-input GAT forward on 8 NeuronCores; returns [50000, 64] f32."""
    global LAST_EXEC_NS, LAST_TRACE, LAST_RES
    from concourse.bass_utils import run_bass_kernel_spmd

    N = 50000
    src = np.asarray(inputs["src"])
    dst = np.asarray(inputs["dst"])

    if "prog" not in _CACHE:
        st = prep_host(src, dst, N, n_cores=8)
        nc = build_program(st)
        _CACHE["prog"] = (st, nc)
    st, nc = _CACHE["prog"]

    in_maps = make_in_maps(st, inputs)
    trace = os.environ.get("GAT_TRACE", "0") == "1"
    res = run_bass_kernel_spmd(nc, in_maps, core_ids=list(range(8)), trace=trace)
    LAST_EXEC_NS = res.exec_time_ns
    LAST_RES = res
    if res.instructions_and_trace:
        LAST_TRACE = res.instructions_and_trace[1]
    return assemble_output(st, [res.results[c] for c in range(8)], N)
